# revision 1
# baseline (speedup 1.0000x reference)
"""Trainium2 Bass kernel for a 2-layer BiLSTM text tagger.

Model (see reference): embedding gather -> BiLSTM(128) -> BiLSTM(128) with
residual -> dense(279) -> softmax. mask_zero=True semantics (state + output
carry-through at masked steps).

Sharding: data-parallel over batch, 4 examples per core on 8 cores. Each core
runs the full network for its slice; no collectives.

Device layout (per core, "layout B" — feature/gate dim on partitions, batch in
the free dim):
  XT[k]  [128, 2048] bf16  - gathered embeddings, transposed; feature = 128k+p,
                             token col j = 4t+e (t-major, e = local example)
  Zb     [128, 16384] bf16 - input projections in PSUM-bank order:
                             col = 32s + 16d + 4c + e (s step, d dir, c gate
                             chunk i/f/g/o, e example). g-chunk pre-scaled by 2
                             so one Sigmoid over all 32 cols computes i,f,o
                             sigmoids and sigma(2 z_g) (tanh via 2*sig(2x)-1).
  H*     [128, 2048] bf16  - hidden states, col = 4t + e
  Recurrence step: one identity-matmul injects 16 steps of Z into a PSUM bank
  (start=True), then per step 8 accumulating matmuls add h @ Wr per
  (dir, gate-chunk); Sigmoid reads the 32-col slice; DVE computes the cell
  update with a fused scalar_tensor_tensor for the tanh fix-up.
"""

import json

import ml_dtypes
import numpy as np

# ---------------------------------------------------------------------------
# problem constants (hardcoded per the contract)
B, T = 32, 512
EMB, UNITS, NCLS = 300, 128, 279
VOCAB = 100000
NCORES = 8
BL = B // NCORES          # 4 examples / core
NTOK = BL * T             # 2048 tokens / core
G4 = 4 * UNITS            # 512
KPAD = 384                # padded embedding dim (3 x 128)
NU = 2048                 # compact table rows (fixed shape across cores)
NTILE = NTOK // 128       # 16 token tiles

_prog_cache = {}


# ---------------------------------------------------------------------------
def _apply_bir_wait_split(bass_mod):
    """This container's walrus rejects >1 sync-wait per instruction. Split
    extras onto inserted EventSemaphore instructions (same engine, in order).
    """
    if getattr(bass_mod.Bass, "_wait_split_applied", False):
        return
    orig = bass_mod.Bass.to_json_bytes
    ctr = [0]

    def fix_list(lst):
        out, changed = [], False
        for ins in lst:
            si = ins.get("sync_info") if isinstance(ins, dict) else None
            if not si:
                out.append(ins)
                continue
            waits = si.get("on_wait") or []
            upds = si.get("on_update") or []
            if len(waits) > 1:
                for w in waits[1:]:
                    ctr[0] += 1
                    out.append({
                        "debug": ins.get("debug", 0), "engine": ins["engine"],
                        "ins": [], "name": f"I-waitfix-{ctr[0]}",
                        "opcode": "EventSemaphore", "outs": [],
                        "sync_info": {"on_update": [], "on_wait": [w]},
                    })
                si["on_wait"] = waits[:1]
                changed = True
            out.append(ins)
            if len(upds) > 1:
                for u in upds[1:]:
                    ctr[0] += 1
                    out.append({
                        "debug": ins.get("debug", 0), "engine": ins["engine"],
                        "ins": [], "name": f"I-updfix-{ctr[0]}",
                        "opcode": "EventSemaphore", "outs": [],
                        "sync_info": {"on_update": [u], "on_wait": []},
                    })
                si["on_update"] = upds[:1]
                changed = True
        return out, changed

    def walk(o):
        if isinstance(o, dict):
            for k, v in o.items():
                if (isinstance(v, list) and v
                        and all(isinstance(e, dict) and "opcode" in e for e in v)):
                    fixed, changed = fix_list(v)
                    if changed:
                        o[k] = fixed
                    for e in o[k]:
                        walk(e)
                else:
                    walk(v)
        elif isinstance(o, list):
            for v in o:
                walk(v)

    def to_json_bytes_fixed(self):
        d = json.loads(orig(self))
        walk(d)
        return json.dumps(d).encode()

    bass_mod.Bass.to_json_bytes = to_json_bytes_fixed
    bass_mod.Bass._wait_split_applied = True


# ---------------------------------------------------------------------------
def _build_program(mask_entries, has_clsb, phases='full', variant=4):
    """Build the Bass program (shared by all 8 cores).

    mask_entries: sorted tuple of (d, s) recurrence slots that need the
    data-driven carry-through lerp (d: 0 fwd / 1 bwd, s: step index).
    """
    import concourse.bass as bass
    import concourse.mybir as mybir
    import concourse.tile as tile

    _apply_bir_wait_split(bass)

    bf16 = mybir.dt.bfloat16
    f32 = mybir.dt.float32
    i32 = mybir.dt.int32
    AF = mybir.ActivationFunctionType
    ALU = mybir.AluOpType

    nc = bass.Bass()

    # ---- DRAM I/O ----
    tbl = nc.dram_tensor("tbl", [NU, KPAD], bf16, kind="ExternalInput")
    idx = nc.dram_tensor("idx", [128, NTILE], i32, kind="ExternalInput")
    ident_d = nc.dram_tensor("ident", [128, 128], bf16, kind="ExternalInput")
    w0_d = nc.dram_tensor("w0", [2, 3, 128, G4], bf16, kind="ExternalInput")
    r0_d = nc.dram_tensor("r0", [2, 128, G4], bf16, kind="ExternalInput")
    w1_d = nc.dram_tensor("w1", [2, 2, 128, G4], bf16, kind="ExternalInput")
    r1_d = nc.dram_tensor("r1", [2, 128, G4], bf16, kind="ExternalInput")
    b0_d = nc.dram_tensor("b0", [128, 8], f32, kind="ExternalInput")
    b1_d = nc.dram_tensor("b1", [128, 8], f32, kind="ExternalInput")
    clsw_d = nc.dram_tensor("clsw", [2, 128, NCLS], bf16, kind="ExternalInput")
    nmask = max(1, len(mask_entries))
    msk_d = nc.dram_tensor("msk", [128, 4 * nmask], f32, kind="ExternalInput")
    clsb_d = None
    if has_clsb:
        clsb_d = nc.dram_tensor("clsb", [128, NCLS], f32, kind="ExternalInput")
    out_d = nc.dram_tensor("out", [NTOK, NCLS], f32, kind="ExternalOutput")

    mask_idx = {ds: i for i, ds in enumerate(mask_entries)}

    with tile.TileContext(nc) as tc:
        with (
            tc.tile_pool(name="const", bufs=1) as cpool,
            tc.tile_pool(name="big", bufs=1) as bigpool,
            tc.tile_pool(name="state", bufs=1) as spool,
        ):
            # ---- constants to SBUF ----
            idx_sb = cpool.tile([128, NTILE], i32)
            nc.gpsimd.dma_start(out=idx_sb[:, :], in_=idx[:, :])
            ident = cpool.tile([128, 128], bf16)
            nc.gpsimd.dma_start(out=ident[:, :], in_=ident_d[:, :])
            w0 = cpool.tile([128, 2, 3, G4], bf16)
            nc.gpsimd.dma_start(
                out=w0[:, :, :, :], in_=w0_d.rearrange("d k p g -> p d k g"))
            r0 = cpool.tile([128, 2, G4], bf16)
            nc.gpsimd.dma_start(out=r0[:, :, :], in_=r0_d.rearrange("d p g -> p d g"))
            w1 = cpool.tile([128, 2, 2, G4], bf16)
            nc.gpsimd.dma_start(
                out=w1[:, :, :, :], in_=w1_d.rearrange("d k p g -> p d k g"))
            r1 = cpool.tile([128, 2, G4], bf16)
            nc.gpsimd.dma_start(out=r1[:, :, :], in_=r1_d.rearrange("d p g -> p d g"))
            b0 = cpool.tile([128, 8], f32)
            nc.gpsimd.dma_start(out=b0[:, :], in_=b0_d[:, :])
            b1 = cpool.tile([128, 8], f32)
            nc.gpsimd.dma_start(out=b1[:, :], in_=b1_d[:, :])
            clsw = cpool.tile([128, 2, NCLS], bf16)
            nc.gpsimd.dma_start(out=clsw[:, :, :], in_=clsw_d.rearrange("k p n -> p k n"))
            msk = cpool.tile([128, 4 * nmask], f32)
            nc.gpsimd.dma_start(out=msk[:, :], in_=msk_d[:, :])
            clsb = None
            if has_clsb:
                clsb = cpool.tile([128, NCLS], f32)
                nc.gpsimd.dma_start(out=clsb[:, :], in_=clsb_d[:, :])

            # ---- big persistent buffers ----
            xt = [bigpool.tile([128, NTOK], bf16, tag=f"xt{k}", name=f"xt{k}")
                  for k in range(3)]
            zb = bigpool.tile([128, 32 * T], bf16)
            h0f = bigpool.tile([128, NTOK], bf16)
            h0b = bigpool.tile([128, NTOK], bf16)
            h1f = bigpool.tile([128, NTOK], bf16)
            h1b = bigpool.tile([128, NTOK], bf16)

            hz = spool.tile([128, 8], bf16)
            nc.vector.memset(hz[:, :], 0.0)

            def strided(tileap, offset, dims):
                return bass.AP(tensor=tileap.tensor, offset=tileap.offset + offset,
                               ap=[tileap.ap[0]] + dims)

            # ================= Phase A: gather + transpose =================
            with (
                tc.tile_pool(name="xrow", bufs=4) as xrow_pool,
                tc.tile_pool(name="tpps", bufs=4, space="PSUM") as tp_pool,
            ):
                for c in range(NTILE):
                    xrow = xrow_pool.tile([128, KPAD], bf16, tag="xrow")
                    nc.gpsimd.indirect_dma_start(
                        out=xrow[:, :], out_offset=None, in_=tbl[:, :],
                        in_offset=bass.IndirectOffsetOnAxis(
                            ap=idx_sb[:, c:c + 1], axis=0),
                    )
                    for k in range(3):
                        pst = tp_pool.tile([128, 128], bf16, tag="tp")
                        nc.tensor.transpose(
                            out=pst[:, :], in_=xrow[:, k * 128:(k + 1) * 128],
                            identity=ident[:, :])
                        nc.vector.tensor_copy(
                            xt[k][:, c * 128:(c + 1) * 128], pst[:, :])

            # ================= shared phase helpers =================
            def projection(layer):
                """Compute Zb for `layer` from its inputs (XT or H0)."""
                w = w0 if layer == 0 else w1
                bia = b0 if layer == 0 else b1
                nk = 3 if layer == 0 else 2
                with tc.tile_pool(name=f"pj{layer}", bufs=4, space="PSUM") as pjp:
                    for d in range(2):
                        for c in range(4):
                            for nb in range(4):
                                ps = pjp.tile([128, 512], f32, tag="pj")
                                s0 = 128 * nb
                                for k in range(nk):
                                    if layer == 0:
                                        src = xt[k][:, :]
                                    else:
                                        src = (h0f if k == 0 else h0b)[:, :]
                                    if d == 0:
                                        rhs = strided(src, 4 * s0,
                                                      [[4, 128], [1, 4]])
                                    else:
                                        rhs = strided(src, 4 * (511 - s0),
                                                      [[-4, 128], [1, 4]])
                                    nc.tensor.matmul(
                                        ps[:, :],
                                        w[:, d, k, c * 128:(c + 1) * 128],
                                        rhs, start=(k == 0), stop=(k == nk - 1))
                                dst = strided(zb[:, :], 32 * s0 + 16 * d + 4 * c,
                                              [[32, 128], [1, 4]])
                                nc.scalar.activation(
                                    dst, ps[:, :], AF.Identity,
                                    bias=bia[:, 4 * d + c:4 * d + c + 1], scale=1.0)

            def recurrence(layer):
                r = r0 if layer == 0 else r1
                Hf = h0f if layer == 0 else h1f
                Hb = h0b if layer == 0 else h1b
                with (
                    tc.tile_pool(name=f"rc{layer}", bufs=4 if variant == 0 else 6,
                                 space="PSUM") as rcp,
                    tc.tile_pool(name=f"gt{layer}", bufs=4 if variant == 0 else 8) as gtp,
                    tc.tile_pool(name=f"tm{layer}", bufs=3 if variant == 0 else 8) as tmp,
                ):
                    c_state = spool.tile([128, 8], f32, tag=f"c{layer}")
                    nc.vector.memset(c_state[:, :], 0.0)
                    ps = None
                    prev_ht = None
                    for s in range(T):
                        sb = s % 16
                        if sb == 0:
                            ps = rcp.tile([128, 512], f32, tag="bank")
                            nc.tensor.matmul(
                                ps[:, :], ident[:, :],
                                zb[:, 512 * (s // 16):512 * (s // 16) + 512],
                                start=True, stop=False, skip_group_check=True)
                        for d in range(2):
                            if s == 0:
                                hprev = hz[:, 4 * d:4 * d + 4]
                            elif variant >= 4 and prev_ht is not None:
                                hprev = prev_ht[:, 4 * d:4 * d + 4]
                            elif d == 0:
                                hprev = Hf[:, 4 * (s - 1):4 * (s - 1) + 4]
                            else:
                                hprev = Hb[:, 4 * (512 - s):4 * (512 - s) + 4]
                            for c in range(4):
                                nc.tensor.matmul(
                                    ps[:, 32 * sb + 16 * d + 4 * c:
                                       32 * sb + 16 * d + 4 * c + 4],
                                    r[:, d, c * 128:(c + 1) * 128],
                                    hprev, start=False, stop=False,
                                    skip_group_check=True)
                        sg = gtp.tile([128, 32], f32, tag="sg")
                        nc.scalar.activation(
                            sg[:, :], ps[:, 32 * sb:32 * sb + 32], AF.Sigmoid)
                        sga = sg[:, :]
                        i_ap = strided(sga, 0, [[16, 2], [1, 4]])
                        f_ap = strided(sga, 4, [[16, 2], [1, 4]])
                        g_ap = strided(sga, 8, [[16, 2], [1, 4]])
                        # u = i*g' ; w = 2u - i ; v = f*c ; c = v + w
                        if variant >= 3:
                            # i*(2g'-1) = 2*i*(g'-0.5): one fused op, then the
                            # *2 folds into the final accumulate.
                            w_t = tmp.tile([128, 8], f32, tag="w")
                            nc.vector.scalar_tensor_tensor(
                                out=w_t[:, :], in0=g_ap, scalar=0.5, in1=i_ap,
                                op0=ALU.subtract, op1=ALU.mult)
                        else:
                            ueng = nc.gpsimd if variant >= 2 else nc.vector
                            u = tmp.tile([128, 8], f32, tag="u")
                            ueng.tensor_tensor(
                                out=u[:, :], in0=i_ap, in1=g_ap, op=ALU.mult)
                            w_t = tmp.tile([128, 8], f32, tag="w")
                            ueng.scalar_tensor_tensor(
                                out=w_t[:, :], in0=u[:, :], scalar=2.0, in1=i_ap,
                                op0=ALU.mult, op1=ALU.subtract)
                        v = tmp.tile([128, 8], f32, tag="v")
                        nc.vector.tensor_tensor(
                            out=v[:, :], in0=f_ap, in1=c_state[:, :], op=ALU.mult)
                        masked = [d for d in range(2) if (d, s) in mask_idx]
                        if not masked:
                            if variant >= 3:
                                nc.vector.scalar_tensor_tensor(
                                    out=c_state[:, :], in0=w_t[:, :], scalar=2.0,
                                    in1=v[:, :], op0=ALU.mult, op1=ALU.add)
                            else:
                                nc.vector.tensor_tensor(
                                    out=c_state[:, :], in0=v[:, :], in1=w_t[:, :],
                                    op=ALU.add)
                            th = tmp.tile([128, 8], f32, tag="th")
                            nc.scalar.activation(th[:, :], c_state[:, :], AF.Tanh)
                            if variant >= 4:
                                o_ap = strided(sga, 12, [[16, 2], [1, 4]])
                                ht = tmp.tile([128, 8], bf16, tag="ht")
                                nc.vector.tensor_tensor(
                                    out=ht[:, :], in0=o_ap, in1=th[:, :],
                                    op=ALU.mult)
                                nc.vector.tensor_copy(
                                    Hf[:, 4 * s:4 * s + 4], ht[:, 0:4])
                                nc.vector.tensor_copy(
                                    Hb[:, 4 * (511 - s):4 * (511 - s) + 4],
                                    ht[:, 4:8])
                                prev_ht = ht
                            else:
                                nc.vector.tensor_tensor(
                                    out=Hf[:, 4 * s:4 * s + 4], in0=sg[:, 12:16],
                                    in1=th[:, 0:4], op=ALU.mult)
                                nc.vector.tensor_tensor(
                                    out=Hb[:, 4 * (511 - s):4 * (511 - s) + 4],
                                    in0=sg[:, 28:32], in1=th[:, 4:8], op=ALU.mult)
                        else:
                            cc = tmp.tile([128, 8], f32, tag="cc")
                            if variant >= 3:
                                nc.vector.scalar_tensor_tensor(
                                    out=cc[:, :], in0=w_t[:, :], scalar=2.0,
                                    in1=v[:, :], op0=ALU.mult, op1=ALU.add)
                            else:
                                nc.vector.tensor_tensor(
                                    out=cc[:, :], in0=v[:, :], in1=w_t[:, :], op=ALU.add)
                            # c lerp: cc_d = c_old + m*(cc_d - c_old)
                            for d in masked:
                                mi = mask_idx[(d, s)]
                                mcol = msk[:, 4 * mi:4 * mi + 4]
                                dd = tmp.tile([128, 4], f32, tag="dd")
                                nc.vector.tensor_tensor(
                                    out=dd[:, :], in0=cc[:, 4 * d:4 * d + 4],
                                    in1=c_state[:, 4 * d:4 * d + 4], op=ALU.subtract)
                                nc.vector.tensor_tensor(
                                    out=dd[:, :], in0=dd[:, :], in1=mcol, op=ALU.mult)
                                nc.vector.tensor_tensor(
                                    out=cc[:, 4 * d:4 * d + 4], in0=dd[:, :],
                                    in1=c_state[:, 4 * d:4 * d + 4], op=ALU.add)
                            nc.vector.tensor_copy(c_state[:, :], cc[:, :])
                            th = tmp.tile([128, 8], f32, tag="th")
                            nc.scalar.activation(th[:, :], c_state[:, :], AF.Tanh)
                            for d in range(2):
                                o_sl = sg[:, 16 * d + 12:16 * d + 16]
                                th_sl = th[:, 4 * d:4 * d + 4]
                                dst = (Hf[:, 4 * s:4 * s + 4] if d == 0 else
                                       Hb[:, 4 * (511 - s):4 * (511 - s) + 4])
                                if d in masked:
                                    mi = mask_idx[(d, s)]
                                    mcol = msk[:, 4 * mi:4 * mi + 4]
                                    if s == 0:
                                        hp = hz[:, 4 * d:4 * d + 4]
                                    elif d == 0:
                                        hp = Hf[:, 4 * (s - 1):4 * (s - 1) + 4]
                                    else:
                                        hp = Hb[:, 4 * (512 - s):4 * (512 - s) + 4]
                                    hn = tmp.tile([128, 4], f32, tag="hn")
                                    nc.vector.tensor_tensor(
                                        out=hn[:, :], in0=o_sl, in1=th_sl,
                                        op=ALU.mult)
                                    nc.vector.tensor_tensor(
                                        out=hn[:, :], in0=hn[:, :], in1=hp,
                                        op=ALU.subtract)
                                    nc.vector.tensor_tensor(
                                        out=hn[:, :], in0=hn[:, :], in1=mcol,
                                        op=ALU.mult)
                                    nc.vector.tensor_tensor(
                                        out=dst, in0=hn[:, :], in1=hp, op=ALU.add)
                                else:
                                    nc.vector.tensor_tensor(
                                        out=dst, in0=o_sl, in1=th_sl, op=ALU.mult)
                            prev_ht = None

            # ================= run the phases =================
            if phases in ('B', 'C', 'full'):
                projection(0)
            if phases in ('C', 'full'):
                recurrence(0)
            if phases == 'full':
                projection(1)
                recurrence(1)

            # ================= classifier + softmax =================
            with (
                tc.tile_pool(name="cls", bufs=4) as clp,
                tc.tile_pool(name="clps", bufs=4, space="PSUM") as clps,
            ):
                for tt in range(NTILE if phases == 'full' else 1):
                    sl = slice(128 * tt, 128 * (tt + 1))
                    i0 = clp.tile([128, 128], bf16, tag="i0")
                    nc.vector.tensor_tensor(
                        out=i0[:, :], in0=h0f[:, sl], in1=h1f[:, sl], op=ALU.add)
                    i1 = clp.tile([128, 128], bf16, tag="i1")
                    nc.vector.tensor_tensor(
                        out=i1[:, :], in0=h0b[:, sl], in1=h1b[:, sl], op=ALU.add)
                    pc = clps.tile([128, NCLS], f32, tag="pc")
                    nc.tensor.matmul(pc[:, :], i0[:, :], clsw[:, 0, :],
                                     start=True, stop=False)
                    nc.tensor.matmul(pc[:, :], i1[:, :], clsw[:, 1, :],
                                     start=False, stop=True)
                    ex = clp.tile([128, NCLS], f32, tag="ex")
                    if has_clsb:
                        nc.vector.tensor_tensor(
                            out=ex[:, :], in0=pc[:, :], in1=clsb[:, :], op=ALU.add)
                        nc.scalar.activation(ex[:, :], ex[:, :], AF.Exp)
                    else:
                        nc.scalar.activation(ex[:, :], pc[:, :], AF.Exp)
                    ssum = clp.tile([128, 1], f32, tag="ss")
                    nc.vector.tensor_reduce(
                        out=ssum[:, :], in_=ex[:, :], op=ALU.add,
                        axis=mybir.AxisListType.X)
                    rec_t = clp.tile([128, 1], f32, tag="rc")
                    nc.vector.reciprocal(rec_t[:, :], ssum[:, :])
                    sm = clp.tile([128, NCLS], f32, tag="sm")
                    nc.vector.tensor_scalar_mul(sm[:, :], ex[:, :], rec_t[:, :])
                    nc.gpsimd.dma_start(out=out_d[sl, :], in_=sm[:, :])

    return nc


# ---------------------------------------------------------------------------
def _prep_host(inputs):
    """Shard + pre-arrange all device inputs. Returns (in_maps, mask_entries,
    has_clsb)."""
    ids = np.asarray(inputs["ids"])
    emb = np.asarray(inputs["emb_table"], dtype=np.float32)

    def gate2(wk):
        w = np.array(wk, dtype=np.float32, copy=True)
        w[:, 2 * UNITS:3 * UNITS] *= 2.0
        return w

    def pad_k(w, kpad):
        out = np.zeros((kpad, G4), np.float32)
        out[:w.shape[0]] = w
        return out

    w0 = np.stack([
        pad_k(gate2(inputs["fw0_k"]), KPAD).reshape(3, 128, G4),
        pad_k(gate2(inputs["bw0_k"]), KPAD).reshape(3, 128, G4),
    ]).astype(ml_dtypes.bfloat16)
    r0 = np.stack([gate2(inputs["fw0_r"]), gate2(inputs["bw0_r"])]
                  ).astype(ml_dtypes.bfloat16)
    w1 = np.stack([
        gate2(inputs["fw1_k"]).reshape(2, 128, G4),
        gate2(inputs["bw1_k"]).reshape(2, 128, G4),
    ]).astype(ml_dtypes.bfloat16)
    r1 = np.stack([gate2(inputs["fw1_r"]), gate2(inputs["bw1_r"])]
                  ).astype(ml_dtypes.bfloat16)

    def bias_tile(bf, bb):
        out = np.zeros((128, 8), np.float32)
        for d, b in enumerate((bf, bb)):
            b = np.array(b, dtype=np.float32, copy=True)
            b[2 * UNITS:3 * UNITS] *= 2.0
            out[:, 4 * d:4 * d + 4] = b.reshape(4, 128).T
        return out

    b0 = bias_tile(inputs["fw0_b"], inputs["bw0_b"])
    b1 = bias_tile(inputs["fw1_b"], inputs["bw1_b"])
    clsw = np.asarray(inputs["cls_w"], np.float32).reshape(2, 128, NCLS).astype(
        ml_dtypes.bfloat16)
    clsb_np = np.asarray(inputs["cls_b"], np.float32)
    has_clsb = bool(np.any(clsb_np != 0))
    ident = np.eye(128, dtype=ml_dtypes.bfloat16)

    mask_entry_set = set()
    per_core = []
    for c in range(NCORES):
        ids_c = ids[BL * c:BL * (c + 1)].astype(np.int64)      # [BL, T]
        ids_tm = ids_c.T.reshape(-1)                           # j = t*BL + e
        uniq, inv = np.unique(ids_tm, return_inverse=True)
        tblp = np.zeros((NU, KPAD), ml_dtypes.bfloat16)
        tblp[:len(uniq), :EMB] = emb[uniq].astype(ml_dtypes.bfloat16)
        idx_np = inv.astype(np.int32).reshape(NTILE, 128).T.copy()
        mask_c = (ids_c != 0)
        for e, t in zip(*np.nonzero(~mask_c)):
            mask_entry_set.add((0, int(t)))          # fwd step s = t
            mask_entry_set.add((1, int(511 - t)))    # bwd step s = 511 - t
        per_core.append((tblp, idx_np, mask_c))

    mask_entries = tuple(sorted(mask_entry_set))
    nmask = max(1, len(mask_entries))

    in_maps = []
    for c in range(NCORES):
        tblp, idx_np, mask_c = per_core[c]
        msk = np.ones((128, 4 * nmask), np.float32)
        for mi, (d, s) in enumerate(mask_entries):
            t = s if d == 0 else 511 - s
            msk[:, 4 * mi:4 * mi + 4] = mask_c[:, t].astype(np.float32)[None, :]
        m = dict(tbl=tblp, idx=idx_np, ident=ident, w0=w0, r0=r0, w1=w1, r1=r1,
                 b0=b0, b1=b1, clsw=clsw, msk=msk)
        if has_clsb:
            m["clsb"] = np.broadcast_to(
                clsb_np.astype(np.float32), (128, NCLS)).copy()
        in_maps.append(m)
    return in_maps, mask_entries, has_clsb


# ---------------------------------------------------------------------------
def kernel(**inputs):
    from concourse.bass_utils import run_bass_kernel_spmd

    in_maps, mask_entries, has_clsb = _prep_host(inputs)

    key = (mask_entries, has_clsb)
    if key not in _prog_cache:
        _prog_cache[key] = _build_program(mask_entries, has_clsb)
    nc = _prog_cache[key]

    res = run_bass_kernel_spmd(nc, in_maps, core_ids=list(range(NCORES)))

    out = np.empty((B, T, NCLS), np.float32)
    for c in range(NCORES):
        oc = res.results[c]["out"].reshape(T, BL, NCLS)
        out[BL * c:BL * (c + 1)] = oc.transpose(1, 0, 2)
    return out



# revision 11
# speedup vs baseline: 14.3593x; 14.3593x over previous
"""Trainium2 Bass kernel for a 2-layer BiLSTM text tagger.

Model (see reference): embedding gather -> BiLSTM(128) -> BiLSTM(128) with
residual -> dense(279) -> softmax. mask_zero=True semantics (state + output
carry-through at masked steps).

Sharding: data-parallel over batch, 4 examples per core on 8 cores. Each core
runs the full network for its slice; no collectives.

Device layout (per core, "layout B" — feature/gate dim on partitions, batch in
the free dim):
  XT[k]  [128, 2048] bf16  - gathered embeddings, transposed; feature = 128k+p,
                             token col j = 4t+e (t-major, e = local example)
  Zb     [128, 16384] bf16 - input projections in PSUM-bank order:
                             col = 32s + 8c + 4d + e (s step, c gate chunk
                             i/f/g/o, d dir, e example) — gate blocks
                             contiguous. g-chunk pre-scaled by 2 so one
                             Sigmoid over all 32 cols computes i,f,o sigmoids
                             and sigma(2 z_g) (tanh via 2*sig(2x)-1).
  H*     [128, 2048] bf16  - hidden states, col = 4t + e
  Recurrence step: one identity-matmul injects 16 steps of Z into a PSUM bank
  (start=True), then per step 8 accumulating matmuls add h @ Wr per
  (dir, gate-chunk); Sigmoid reads the 32-col slice; DVE computes the cell
  update with a fused scalar_tensor_tensor for the tanh fix-up.
"""

import json

import ml_dtypes
import numpy as np

# ---------------------------------------------------------------------------
# problem constants (hardcoded per the contract)
B, T = 32, 512
EMB, UNITS, NCLS = 300, 128, 279
VOCAB = 100000
NCORES = 8
BL = B // NCORES          # 4 examples / core
NTOK = BL * T             # 2048 tokens / core
G4 = 4 * UNITS            # 512
KPAD = 384                # padded embedding dim (3 x 128)
NU = 2048                 # compact table rows (fixed shape across cores)
NTILE = NTOK // 128       # 16 token tiles
VARIANT = 4               # recurrence micro-schedule variant

_prog_cache = {}


# ---------------------------------------------------------------------------
def _apply_bir_wait_split(bass_mod):
    """This container's walrus rejects >1 sync-wait per instruction. Split
    extras onto inserted EventSemaphore instructions (same engine, in order).
    """
    if getattr(bass_mod.Bass, "_wait_split_applied", False):
        return
    orig = bass_mod.Bass.to_json_bytes
    ctr = [0]

    def fix_list(lst):
        out, changed = [], False
        for ins in lst:
            si = ins.get("sync_info") if isinstance(ins, dict) else None
            if not si:
                out.append(ins)
                continue
            waits = si.get("on_wait") or []
            upds = si.get("on_update") or []
            if len(waits) > 1:
                for w in waits[1:]:
                    ctr[0] += 1
                    out.append({
                        "debug": ins.get("debug", 0), "engine": ins["engine"],
                        "ins": [], "name": f"I-waitfix-{ctr[0]}",
                        "opcode": "EventSemaphore", "outs": [],
                        "sync_info": {"on_update": [], "on_wait": [w]},
                    })
                si["on_wait"] = waits[:1]
                changed = True
            out.append(ins)
            if len(upds) > 1:
                for u in upds[1:]:
                    ctr[0] += 1
                    out.append({
                        "debug": ins.get("debug", 0), "engine": ins["engine"],
                        "ins": [], "name": f"I-updfix-{ctr[0]}",
                        "opcode": "EventSemaphore", "outs": [],
                        "sync_info": {"on_update": [u], "on_wait": []},
                    })
                si["on_update"] = upds[:1]
                changed = True
        return out, changed

    def walk(o):
        if isinstance(o, dict):
            for k, v in o.items():
                if (isinstance(v, list) and v
                        and all(isinstance(e, dict) and "opcode" in e for e in v)):
                    fixed, changed = fix_list(v)
                    if changed:
                        o[k] = fixed
                    for e in o[k]:
                        walk(e)
                else:
                    walk(v)
        elif isinstance(o, list):
            for v in o:
                walk(v)

    def to_json_bytes_fixed(self):
        d = json.loads(orig(self))
        walk(d)
        return json.dumps(d).encode()

    bass_mod.Bass.to_json_bytes = to_json_bytes_fixed
    bass_mod.Bass._wait_split_applied = True


# ---------------------------------------------------------------------------
def _build_program(mask_entries, has_clsb, phases='full', variant=4):
    """Build the Bass program (shared by all 8 cores).

    mask_entries: sorted tuple of (d, s) recurrence slots that need the
    data-driven carry-through lerp (d: 0 fwd / 1 bwd, s: step index).
    """
    import concourse.bass as bass
    import concourse.mybir as mybir
    import concourse.tile as tile

    _apply_bir_wait_split(bass)

    bf16 = mybir.dt.bfloat16
    f32 = mybir.dt.float32
    i32 = mybir.dt.int32
    AF = mybir.ActivationFunctionType
    ALU = mybir.AluOpType

    nc = bass.Bass()

    # ---- DRAM I/O ----
    tbl = nc.dram_tensor("tbl", [NU, KPAD], bf16, kind="ExternalInput")
    idx = nc.dram_tensor("idx", [128, NTILE], i32, kind="ExternalInput")
    ident_d = nc.dram_tensor("ident", [128, 128], bf16, kind="ExternalInput")
    w0_d = nc.dram_tensor("w0", [2, 3, 128, G4], bf16, kind="ExternalInput")
    r0_d = nc.dram_tensor("r0", [2, 128, G4], bf16, kind="ExternalInput")
    w1_d = nc.dram_tensor("w1", [2, 2, 128, G4], bf16, kind="ExternalInput")
    r1_d = nc.dram_tensor("r1", [2, 128, G4], bf16, kind="ExternalInput")
    b0_d = nc.dram_tensor("b0", [128, 8], f32, kind="ExternalInput")
    b1_d = nc.dram_tensor("b1", [128, 8], f32, kind="ExternalInput")
    clsw_d = nc.dram_tensor("clsw", [2, 128, NCLS], bf16, kind="ExternalInput")
    nmask = max(1, len(mask_entries))
    msk_d = nc.dram_tensor("msk", [128, 4 * nmask], f32, kind="ExternalInput")
    clsb_d = None
    if has_clsb:
        clsb_d = nc.dram_tensor("clsb", [128, NCLS], f32, kind="ExternalInput")
    out_d = nc.dram_tensor("out", [NTOK, NCLS], f32, kind="ExternalOutput")

    mask_idx = {ds: i for i, ds in enumerate(mask_entries)}

    with tile.TileContext(nc) as tc:
        with (
            tc.tile_pool(name="const", bufs=1) as cpool,
            tc.tile_pool(name="big", bufs=1) as bigpool,
            tc.tile_pool(name="state", bufs=1) as spool,
        ):
            # ---- constants to SBUF ----
            idx_sb = cpool.tile([128, NTILE], i32)
            nc.gpsimd.dma_start(out=idx_sb[:, :], in_=idx[:, :])
            ident = cpool.tile([128, 128], bf16)
            nc.gpsimd.dma_start(out=ident[:, :], in_=ident_d[:, :])
            w0 = cpool.tile([128, 2, 3, G4], bf16)
            nc.gpsimd.dma_start(
                out=w0[:, :, :, :], in_=w0_d.rearrange("d k p g -> p d k g"))
            r0 = cpool.tile([128, 2, G4], bf16)
            nc.gpsimd.dma_start(out=r0[:, :, :], in_=r0_d.rearrange("d p g -> p d g"))
            w1 = cpool.tile([128, 2, 2, G4], bf16)
            nc.gpsimd.dma_start(
                out=w1[:, :, :, :], in_=w1_d.rearrange("d k p g -> p d k g"))
            r1 = cpool.tile([128, 2, G4], bf16)
            nc.gpsimd.dma_start(out=r1[:, :, :], in_=r1_d.rearrange("d p g -> p d g"))
            b0 = cpool.tile([128, 8], f32)
            nc.gpsimd.dma_start(out=b0[:, :], in_=b0_d[:, :])
            b1 = cpool.tile([128, 8], f32)
            nc.gpsimd.dma_start(out=b1[:, :], in_=b1_d[:, :])
            clsw = cpool.tile([128, 2, NCLS], bf16)
            nc.gpsimd.dma_start(out=clsw[:, :, :], in_=clsw_d.rearrange("k p n -> p k n"))
            msk = cpool.tile([128, 4 * nmask], f32)
            nc.gpsimd.dma_start(out=msk[:, :], in_=msk_d[:, :])
            clsb = None
            if has_clsb:
                clsb = cpool.tile([128, NCLS], f32)
                nc.gpsimd.dma_start(out=clsb[:, :], in_=clsb_d[:, :])

            # ---- big persistent buffers ----
            xt = [bigpool.tile([128, NTOK], bf16, tag=f"xt{k}", name=f"xt{k}")
                  for k in range(3)]
            zb = bigpool.tile([128, 32 * T], bf16)
            h0f = bigpool.tile([128, NTOK], bf16)
            h0b = bigpool.tile([128, NTOK], bf16)
            h1f = bigpool.tile([128, NTOK], bf16)
            h1b = bigpool.tile([128, NTOK], bf16)

            hz = spool.tile([128, 8], bf16)
            nc.vector.memset(hz[:, :], 0.0)

            def strided(tileap, offset, dims):
                return bass.AP(tensor=tileap.tensor, offset=tileap.offset + offset,
                               ap=[tileap.ap[0]] + dims)

            # ================= Phase A: gather + transpose =================
            with (
                tc.tile_pool(name="xrow", bufs=4) as xrow_pool,
                tc.tile_pool(name="tpps", bufs=4, space="PSUM") as tp_pool,
            ):
                for c in range(NTILE):
                    xrow = xrow_pool.tile([128, KPAD], bf16, tag="xrow")
                    nc.gpsimd.indirect_dma_start(
                        out=xrow[:, :], out_offset=None, in_=tbl[:, :],
                        in_offset=bass.IndirectOffsetOnAxis(
                            ap=idx_sb[:, c:c + 1], axis=0),
                    )
                    for k in range(3):
                        pst = tp_pool.tile([128, 128], bf16, tag="tp")
                        nc.tensor.transpose(
                            out=pst[:, :], in_=xrow[:, k * 128:(k + 1) * 128],
                            identity=ident[:, :])
                        nc.vector.tensor_copy(
                            xt[k][:, c * 128:(c + 1) * 128], pst[:, :])

            # ================= shared phase helpers =================
            def projection(layer):
                """Compute Zb for `layer` from its inputs (XT or H0)."""
                w = w0 if layer == 0 else w1
                bia = b0 if layer == 0 else b1
                nk = 3 if layer == 0 else 2
                with tc.tile_pool(name=f"pj{layer}", bufs=4, space="PSUM") as pjp:
                    for d in range(2):
                        for c in range(4):
                            for nb in range(4):
                                ps = pjp.tile([128, 512], f32, tag="pj")
                                s0 = 128 * nb
                                for k in range(nk):
                                    if layer == 0:
                                        src = xt[k][:, :]
                                    else:
                                        src = (h0f if k == 0 else h0b)[:, :]
                                    if d == 0:
                                        rhs = strided(src, 4 * s0,
                                                      [[4, 128], [1, 4]])
                                    else:
                                        rhs = strided(src, 4 * (511 - s0),
                                                      [[-4, 128], [1, 4]])
                                    nc.tensor.matmul(
                                        ps[:, :],
                                        w[:, d, k, c * 128:(c + 1) * 128],
                                        rhs, start=(k == 0), stop=(k == nk - 1))
                                dst = strided(zb[:, :], 32 * s0 + 8 * c + 4 * d,
                                              [[32, 128], [1, 4]])
                                nc.scalar.activation(
                                    dst, ps[:, :], AF.Identity,
                                    bias=bia[:, 4 * d + c:4 * d + c + 1], scale=1.0)

            def recurrence(layer):
                r = r0 if layer == 0 else r1
                Hf = h0f if layer == 0 else h1f
                Hb = h0b if layer == 0 else h1b
                with (
                    tc.tile_pool(name=f"rc{layer}", bufs=4 if variant == 0 else 6,
                                 space="PSUM") as rcp,
                    tc.tile_pool(name=f"gt{layer}", bufs=4 if variant == 0 else 8) as gtp,
                    tc.tile_pool(name=f"tm{layer}", bufs=3 if variant == 0 else 8) as tmp,
                ):
                    c_state = spool.tile([128, 8], f32, tag=f"c{layer}")
                    nc.vector.memset(c_state[:, :], 0.0)
                    ps = None
                    prev_ht = None
                    for s in range(T):
                        sb = s % 16
                        if sb == 0:
                            ps = rcp.tile([128, 512], f32, tag="bank")
                            nc.tensor.matmul(
                                ps[:, :], ident[:, :],
                                zb[:, 512 * (s // 16):512 * (s // 16) + 512],
                                start=True, stop=False, skip_group_check=True)
                        for d in range(2):
                            if s == 0:
                                hprev = hz[:, 4 * d:4 * d + 4]
                            elif variant >= 4 and prev_ht is not None:
                                hprev = prev_ht[:, 4 * d:4 * d + 4]
                            elif d == 0:
                                hprev = Hf[:, 4 * (s - 1):4 * (s - 1) + 4]
                            else:
                                hprev = Hb[:, 4 * (512 - s):4 * (512 - s) + 4]
                            for c in range(4):
                                nc.tensor.matmul(
                                    ps[:, 32 * sb + 8 * c + 4 * d:
                                       32 * sb + 8 * c + 4 * d + 4],
                                    r[:, d, c * 128:(c + 1) * 128],
                                    hprev, start=False, stop=False,
                                    skip_group_check=True)
                        sg = gtp.tile([128, 32], f32, tag="sg")
                        nc.scalar.activation(
                            sg[:, :], ps[:, 32 * sb:32 * sb + 32], AF.Sigmoid)
                        # gate blocks are contiguous: col = 8c + 4d + e
                        i_ap = sg[:, 0:8]
                        f_ap = sg[:, 8:16]
                        g_ap = sg[:, 16:24]
                        # u = i*g' ; w = 2u - i ; v = f*c ; c = v + w
                        if variant >= 3:
                            # i*(2g'-1) = 2*i*(g'-0.5): one fused op, then the
                            # *2 folds into the final accumulate.
                            w_t = tmp.tile([128, 8], f32, tag="w")
                            nc.vector.scalar_tensor_tensor(
                                out=w_t[:, :], in0=g_ap, scalar=0.5, in1=i_ap,
                                op0=ALU.subtract, op1=ALU.mult)
                        else:
                            ueng = nc.gpsimd if variant >= 2 else nc.vector
                            u = tmp.tile([128, 8], f32, tag="u")
                            ueng.tensor_tensor(
                                out=u[:, :], in0=i_ap, in1=g_ap, op=ALU.mult)
                            w_t = tmp.tile([128, 8], f32, tag="w")
                            ueng.scalar_tensor_tensor(
                                out=w_t[:, :], in0=u[:, :], scalar=2.0, in1=i_ap,
                                op0=ALU.mult, op1=ALU.subtract)
                        v = tmp.tile([128, 8], f32, tag="v")
                        # variant 5: f*c on GpSimd, in parallel with w on DVE
                        veng = nc.gpsimd if variant >= 5 else nc.vector
                        veng.tensor_tensor(
                            out=v[:, :], in0=f_ap, in1=c_state[:, :], op=ALU.mult)
                        masked = [d for d in range(2) if (d, s) in mask_idx]
                        if not masked:
                            if variant >= 3:
                                nc.vector.scalar_tensor_tensor(
                                    out=c_state[:, :], in0=w_t[:, :], scalar=2.0,
                                    in1=v[:, :], op0=ALU.mult, op1=ALU.add)
                            else:
                                nc.vector.tensor_tensor(
                                    out=c_state[:, :], in0=v[:, :], in1=w_t[:, :],
                                    op=ALU.add)
                            th = tmp.tile([128, 8], f32, tag="th")
                            nc.scalar.activation(th[:, :], c_state[:, :], AF.Tanh)
                            if variant >= 4:
                                o_ap = sg[:, 24:32]
                                ht = tmp.tile([128, 8], bf16, tag="ht")
                                nc.vector.tensor_tensor(
                                    out=ht[:, :], in0=o_ap, in1=th[:, :],
                                    op=ALU.mult)
                                nc.vector.tensor_copy(
                                    Hf[:, 4 * s:4 * s + 4], ht[:, 0:4])
                                nc.vector.tensor_copy(
                                    Hb[:, 4 * (511 - s):4 * (511 - s) + 4],
                                    ht[:, 4:8])
                                prev_ht = ht
                            else:
                                nc.vector.tensor_tensor(
                                    out=Hf[:, 4 * s:4 * s + 4], in0=sg[:, 24:28],
                                    in1=th[:, 0:4], op=ALU.mult)
                                nc.vector.tensor_tensor(
                                    out=Hb[:, 4 * (511 - s):4 * (511 - s) + 4],
                                    in0=sg[:, 28:32], in1=th[:, 4:8], op=ALU.mult)
                        else:
                            cc = tmp.tile([128, 8], f32, tag="cc")
                            if variant >= 3:
                                nc.vector.scalar_tensor_tensor(
                                    out=cc[:, :], in0=w_t[:, :], scalar=2.0,
                                    in1=v[:, :], op0=ALU.mult, op1=ALU.add)
                            else:
                                nc.vector.tensor_tensor(
                                    out=cc[:, :], in0=v[:, :], in1=w_t[:, :], op=ALU.add)
                            # c lerp: cc_d = c_old + m*(cc_d - c_old)
                            for d in masked:
                                mi = mask_idx[(d, s)]
                                mcol = msk[:, 4 * mi:4 * mi + 4]
                                dd = tmp.tile([128, 4], f32, tag="dd")
                                nc.vector.tensor_tensor(
                                    out=dd[:, :], in0=cc[:, 4 * d:4 * d + 4],
                                    in1=c_state[:, 4 * d:4 * d + 4], op=ALU.subtract)
                                nc.vector.tensor_tensor(
                                    out=dd[:, :], in0=dd[:, :], in1=mcol, op=ALU.mult)
                                nc.vector.tensor_tensor(
                                    out=cc[:, 4 * d:4 * d + 4], in0=dd[:, :],
                                    in1=c_state[:, 4 * d:4 * d + 4], op=ALU.add)
                            nc.vector.tensor_copy(c_state[:, :], cc[:, :])
                            th = tmp.tile([128, 8], f32, tag="th")
                            nc.scalar.activation(th[:, :], c_state[:, :], AF.Tanh)
                            for d in range(2):
                                o_sl = sg[:, 24 + 4 * d:28 + 4 * d]
                                th_sl = th[:, 4 * d:4 * d + 4]
                                dst = (Hf[:, 4 * s:4 * s + 4] if d == 0 else
                                       Hb[:, 4 * (511 - s):4 * (511 - s) + 4])
                                if d in masked:
                                    mi = mask_idx[(d, s)]
                                    mcol = msk[:, 4 * mi:4 * mi + 4]
                                    if s == 0:
                                        hp = hz[:, 4 * d:4 * d + 4]
                                    elif d == 0:
                                        hp = Hf[:, 4 * (s - 1):4 * (s - 1) + 4]
                                    else:
                                        hp = Hb[:, 4 * (512 - s):4 * (512 - s) + 4]
                                    hn = tmp.tile([128, 4], f32, tag="hn")
                                    nc.vector.tensor_tensor(
                                        out=hn[:, :], in0=o_sl, in1=th_sl,
                                        op=ALU.mult)
                                    nc.vector.tensor_tensor(
                                        out=hn[:, :], in0=hn[:, :], in1=hp,
                                        op=ALU.subtract)
                                    nc.vector.tensor_tensor(
                                        out=hn[:, :], in0=hn[:, :], in1=mcol,
                                        op=ALU.mult)
                                    nc.vector.tensor_tensor(
                                        out=dst, in0=hn[:, :], in1=hp, op=ALU.add)
                                else:
                                    nc.vector.tensor_tensor(
                                        out=dst, in0=o_sl, in1=th_sl, op=ALU.mult)
                            prev_ht = None

            # ================= run the phases =================
            if phases in ('B', 'C', 'full'):
                projection(0)
            if phases in ('C', 'full'):
                recurrence(0)
            if phases == 'full':
                projection(1)
                recurrence(1)

            # ================= classifier + softmax =================
            with (
                tc.tile_pool(name="cls", bufs=4) as clp,
                tc.tile_pool(name="clps", bufs=4, space="PSUM") as clps,
            ):
                for tt in range(NTILE if phases == 'full' else 1):
                    sl = slice(128 * tt, 128 * (tt + 1))
                    i0 = clp.tile([128, 128], bf16, tag="i0")
                    nc.vector.tensor_tensor(
                        out=i0[:, :], in0=h0f[:, sl], in1=h1f[:, sl], op=ALU.add)
                    i1 = clp.tile([128, 128], bf16, tag="i1")
                    nc.vector.tensor_tensor(
                        out=i1[:, :], in0=h0b[:, sl], in1=h1b[:, sl], op=ALU.add)
                    pc = clps.tile([128, NCLS], f32, tag="pc")
                    nc.tensor.matmul(pc[:, :], i0[:, :], clsw[:, 0, :],
                                     start=True, stop=False)
                    nc.tensor.matmul(pc[:, :], i1[:, :], clsw[:, 1, :],
                                     start=False, stop=True)
                    ex = clp.tile([128, NCLS], f32, tag="ex")
                    if has_clsb:
                        nc.vector.tensor_tensor(
                            out=ex[:, :], in0=pc[:, :], in1=clsb[:, :], op=ALU.add)
                        nc.scalar.activation(ex[:, :], ex[:, :], AF.Exp)
                    else:
                        nc.scalar.activation(ex[:, :], pc[:, :], AF.Exp)
                    ssum = clp.tile([128, 1], f32, tag="ss")
                    nc.vector.tensor_reduce(
                        out=ssum[:, :], in_=ex[:, :], op=ALU.add,
                        axis=mybir.AxisListType.X)
                    rec_t = clp.tile([128, 1], f32, tag="rc")
                    nc.vector.reciprocal(rec_t[:, :], ssum[:, :])
                    sm = clp.tile([128, NCLS], f32, tag="sm")
                    nc.vector.tensor_scalar_mul(sm[:, :], ex[:, :], rec_t[:, :])
                    nc.gpsimd.dma_start(out=out_d[sl, :], in_=sm[:, :])

    return nc


# ---------------------------------------------------------------------------
def _prep_host(inputs):
    """Shard + pre-arrange all device inputs. Returns (in_maps, mask_entries,
    has_clsb)."""
    ids = np.asarray(inputs["ids"])
    emb = np.asarray(inputs["emb_table"], dtype=np.float32)

    def gate2(wk):
        w = np.array(wk, dtype=np.float32, copy=True)
        w[:, 2 * UNITS:3 * UNITS] *= 2.0
        return w

    def pad_k(w, kpad):
        out = np.zeros((kpad, G4), np.float32)
        out[:w.shape[0]] = w
        return out

    w0 = np.stack([
        pad_k(gate2(inputs["fw0_k"]), KPAD).reshape(3, 128, G4),
        pad_k(gate2(inputs["bw0_k"]), KPAD).reshape(3, 128, G4),
    ]).astype(ml_dtypes.bfloat16)
    r0 = np.stack([gate2(inputs["fw0_r"]), gate2(inputs["bw0_r"])]
                  ).astype(ml_dtypes.bfloat16)
    w1 = np.stack([
        gate2(inputs["fw1_k"]).reshape(2, 128, G4),
        gate2(inputs["bw1_k"]).reshape(2, 128, G4),
    ]).astype(ml_dtypes.bfloat16)
    r1 = np.stack([gate2(inputs["fw1_r"]), gate2(inputs["bw1_r"])]
                  ).astype(ml_dtypes.bfloat16)

    def bias_tile(bf, bb):
        out = np.zeros((128, 8), np.float32)
        for d, b in enumerate((bf, bb)):
            b = np.array(b, dtype=np.float32, copy=True)
            b[2 * UNITS:3 * UNITS] *= 2.0
            out[:, 4 * d:4 * d + 4] = b.reshape(4, 128).T
        return out

    b0 = bias_tile(inputs["fw0_b"], inputs["bw0_b"])
    b1 = bias_tile(inputs["fw1_b"], inputs["bw1_b"])
    clsw = np.asarray(inputs["cls_w"], np.float32).reshape(2, 128, NCLS).astype(
        ml_dtypes.bfloat16)
    clsb_np = np.asarray(inputs["cls_b"], np.float32)
    has_clsb = bool(np.any(clsb_np != 0))
    ident = np.eye(128, dtype=ml_dtypes.bfloat16)

    mask_entry_set = set()
    per_core = []
    for c in range(NCORES):
        ids_c = ids[BL * c:BL * (c + 1)].astype(np.int64)      # [BL, T]
        ids_tm = ids_c.T.reshape(-1)                           # j = t*BL + e
        uniq, inv = np.unique(ids_tm, return_inverse=True)
        tblp = np.zeros((NU, KPAD), ml_dtypes.bfloat16)
        tblp[:len(uniq), :EMB] = emb[uniq].astype(ml_dtypes.bfloat16)
        idx_np = inv.astype(np.int32).reshape(NTILE, 128).T.copy()
        mask_c = (ids_c != 0)
        for e, t in zip(*np.nonzero(~mask_c)):
            mask_entry_set.add((0, int(t)))          # fwd step s = t
            mask_entry_set.add((1, int(511 - t)))    # bwd step s = 511 - t
        per_core.append((tblp, idx_np, mask_c))

    mask_entries = tuple(sorted(mask_entry_set))
    nmask = max(1, len(mask_entries))

    in_maps = []
    for c in range(NCORES):
        tblp, idx_np, mask_c = per_core[c]
        msk = np.ones((128, 4 * nmask), np.float32)
        for mi, (d, s) in enumerate(mask_entries):
            t = s if d == 0 else 511 - s
            msk[:, 4 * mi:4 * mi + 4] = mask_c[:, t].astype(np.float32)[None, :]
        m = dict(tbl=tblp, idx=idx_np, ident=ident, w0=w0, r0=r0, w1=w1, r1=r1,
                 b0=b0, b1=b1, clsw=clsw, msk=msk)
        if has_clsb:
            m["clsb"] = np.broadcast_to(
                clsb_np.astype(np.float32), (128, NCLS)).copy()
        in_maps.append(m)
    return in_maps, mask_entries, has_clsb


# ---------------------------------------------------------------------------
def kernel(**inputs):
    from concourse.bass_utils import run_bass_kernel_spmd

    in_maps, mask_entries, has_clsb = _prep_host(inputs)

    key = (mask_entries, has_clsb, VARIANT)
    if key not in _prog_cache:
        _prog_cache[key] = _build_program(mask_entries, has_clsb,
                                          variant=VARIANT)
    nc = _prog_cache[key]

    res = run_bass_kernel_spmd(nc, in_maps, core_ids=list(range(NCORES)))

    out = np.empty((B, T, NCLS), np.float32)
    for c in range(NCORES):
        oc = res.results[c]["out"].reshape(T, BL, NCLS)
        out[BL * c:BL * (c + 1)] = oc.transpose(1, 0, 2)
    return out



# revision 17
# speedup vs baseline: 14.4675x; 1.0075x over previous
"""Trainium2 Bass kernel for a 2-layer BiLSTM text tagger.

Model (see reference): embedding gather -> BiLSTM(128) -> BiLSTM(128) with
residual -> dense(279) -> softmax. mask_zero=True semantics (state + output
carry-through at masked steps).

Sharding: data-parallel over batch, 4 examples per core on 8 cores. Each core
runs the full network for its slice; no collectives.

Device layout (per core, "layout B" — feature/gate dim on partitions, batch in
the free dim):
  XT[k]  [128, 2048] bf16  - gathered embeddings, transposed; feature = 128k+p,
                             token col j = 4t+e (t-major, e = local example)
  Zb     [128, 16384] bf16 - input projections in PSUM-bank order:
                             col = 32s + 8c + 4d + e (s step, c gate chunk
                             i/f/g/o, d dir, e example) — gate blocks
                             contiguous. g-chunk pre-scaled by 2 so one
                             Sigmoid over all 32 cols computes i,f,o sigmoids
                             and sigma(2 z_g) (tanh via 2*sig(2x)-1).
  H*     [128, 2048] bf16  - hidden states, col = 4t + e
  Recurrence step: one identity-matmul injects 16 steps of Z into a PSUM bank
  (start=True), then per step 8 accumulating matmuls add h @ Wr per
  (dir, gate-chunk); Sigmoid reads the 32-col slice; DVE computes the cell
  update with a fused scalar_tensor_tensor for the tanh fix-up.
"""

import json

import ml_dtypes
import numpy as np

# ---------------------------------------------------------------------------
# problem constants (hardcoded per the contract)
B, T = 32, 512
EMB, UNITS, NCLS = 300, 128, 279
VOCAB = 100000
NCORES = 8
BL = B // NCORES          # 4 examples / core
NTOK = BL * T             # 2048 tokens / core
G4 = 4 * UNITS            # 512
KPAD = 384                # padded embedding dim (3 x 128)
NU = 2048                 # compact table rows (fixed shape across cores)
NTILE = NTOK // 128       # 16 token tiles
VARIANT = 4               # recurrence micro-schedule variant

_prog_cache = {}


# ---------------------------------------------------------------------------
def _apply_bir_wait_split(bass_mod):
    """This container's walrus rejects >1 sync-wait per instruction. Split
    extras onto inserted EventSemaphore instructions (same engine, in order).
    """
    if getattr(bass_mod.Bass, "_wait_split_applied", False):
        return
    orig = bass_mod.Bass.to_json_bytes
    ctr = [0]

    def fix_list(lst):
        out, changed = [], False
        for ins in lst:
            si = ins.get("sync_info") if isinstance(ins, dict) else None
            if not si:
                out.append(ins)
                continue
            waits = si.get("on_wait") or []
            upds = si.get("on_update") or []
            if len(waits) > 1:
                for w in waits[1:]:
                    ctr[0] += 1
                    out.append({
                        "debug": ins.get("debug", 0), "engine": ins["engine"],
                        "ins": [], "name": f"I-waitfix-{ctr[0]}",
                        "opcode": "EventSemaphore", "outs": [],
                        "sync_info": {"on_update": [], "on_wait": [w]},
                    })
                si["on_wait"] = waits[:1]
                changed = True
            out.append(ins)
            if len(upds) > 1:
                for u in upds[1:]:
                    ctr[0] += 1
                    out.append({
                        "debug": ins.get("debug", 0), "engine": ins["engine"],
                        "ins": [], "name": f"I-updfix-{ctr[0]}",
                        "opcode": "EventSemaphore", "outs": [],
                        "sync_info": {"on_update": [u], "on_wait": []},
                    })
                si["on_update"] = upds[:1]
                changed = True
        return out, changed

    def walk(o):
        if isinstance(o, dict):
            for k, v in o.items():
                if (isinstance(v, list) and v
                        and all(isinstance(e, dict) and "opcode" in e for e in v)):
                    fixed, changed = fix_list(v)
                    if changed:
                        o[k] = fixed
                    for e in o[k]:
                        walk(e)
                else:
                    walk(v)
        elif isinstance(o, list):
            for v in o:
                walk(v)

    def to_json_bytes_fixed(self):
        d = json.loads(orig(self))
        walk(d)
        return json.dumps(d).encode()

    bass_mod.Bass.to_json_bytes = to_json_bytes_fixed
    bass_mod.Bass._wait_split_applied = True


# ---------------------------------------------------------------------------
def _build_program(mask_entries, has_clsb, phases='full', variant=4):
    """Build the Bass program (shared by all 8 cores).

    mask_entries: sorted tuple of (d, s) recurrence slots that need the
    data-driven carry-through lerp (d: 0 fwd / 1 bwd, s: step index).
    """
    import concourse.bass as bass
    import concourse.mybir as mybir
    import concourse.tile as tile

    _apply_bir_wait_split(bass)

    bf16 = mybir.dt.bfloat16
    f32 = mybir.dt.float32
    i32 = mybir.dt.int32
    AF = mybir.ActivationFunctionType
    ALU = mybir.AluOpType

    nc = bass.Bass()

    # ---- DRAM I/O ----
    tbl = nc.dram_tensor("tbl", [NU, KPAD], bf16, kind="ExternalInput")
    idx = nc.dram_tensor("idx", [128, NTILE], i32, kind="ExternalInput")
    ident_d = nc.dram_tensor("ident", [128, 128], bf16, kind="ExternalInput")
    w0_d = nc.dram_tensor("w0", [2, 3, 128, G4], bf16, kind="ExternalInput")
    r0_d = nc.dram_tensor("r0", [2, 128, G4], bf16, kind="ExternalInput")
    w1_d = nc.dram_tensor("w1", [2, 2, 128, G4], bf16, kind="ExternalInput")
    r1_d = nc.dram_tensor("r1", [2, 128, G4], bf16, kind="ExternalInput")
    b0_d = nc.dram_tensor("b0", [128, 8], f32, kind="ExternalInput")
    b1_d = nc.dram_tensor("b1", [128, 8], f32, kind="ExternalInput")
    clsw_d = nc.dram_tensor("clsw", [2, 128, NCLS], bf16, kind="ExternalInput")
    nmask = max(1, len(mask_entries))
    msk_d = nc.dram_tensor("msk", [128, 4 * nmask], f32, kind="ExternalInput")
    clsb_d = None
    if has_clsb:
        clsb_d = nc.dram_tensor("clsb", [128, NCLS], f32, kind="ExternalInput")
    out_d = nc.dram_tensor("out", [NTOK, NCLS], f32, kind="ExternalOutput")

    mask_idx = {ds: i for i, ds in enumerate(mask_entries)}

    with tile.TileContext(nc) as tc:
        with (
            tc.tile_pool(name="const", bufs=1) as cpool,
            tc.tile_pool(name="big", bufs=1) as bigpool,
            tc.tile_pool(name="state", bufs=1) as spool,
        ):
            # ---- constants to SBUF ----
            # idx/ident gate the gather phase: keep them first on the gpsimd
            # queue. Everything else (needed only from proj0 onward) goes on
            # the idle sync-engine queue so the indirect gather DMAs are not
            # stuck behind ~7us of constant-DMA issue.
            idx_sb = cpool.tile([128, NTILE], i32)
            nc.gpsimd.dma_start(out=idx_sb[:, :], in_=idx[:, :])
            ident = cpool.tile([128, 128], bf16)
            nc.gpsimd.dma_start(out=ident[:, :], in_=ident_d[:, :])
            w0 = cpool.tile([128, 2, 3, G4], bf16)
            nc.sync.dma_start(
                out=w0[:, :, :, :], in_=w0_d.rearrange("d k p g -> p d k g"))
            r0 = cpool.tile([128, 2, G4], bf16)
            nc.sync.dma_start(out=r0[:, :, :], in_=r0_d.rearrange("d p g -> p d g"))
            w1 = cpool.tile([128, 2, 2, G4], bf16)
            nc.sync.dma_start(
                out=w1[:, :, :, :], in_=w1_d.rearrange("d k p g -> p d k g"))
            r1 = cpool.tile([128, 2, G4], bf16)
            nc.sync.dma_start(out=r1[:, :, :], in_=r1_d.rearrange("d p g -> p d g"))
            b0 = cpool.tile([128, 8], f32)
            nc.sync.dma_start(out=b0[:, :], in_=b0_d[:, :])
            b1 = cpool.tile([128, 8], f32)
            nc.sync.dma_start(out=b1[:, :], in_=b1_d[:, :])
            clsw = cpool.tile([128, 2, NCLS], bf16)
            nc.sync.dma_start(out=clsw[:, :, :], in_=clsw_d.rearrange("k p n -> p k n"))
            msk = cpool.tile([128, 4 * nmask], f32)
            nc.sync.dma_start(out=msk[:, :], in_=msk_d[:, :])
            clsb = None
            if has_clsb:
                clsb = cpool.tile([128, NCLS], f32)
                nc.sync.dma_start(out=clsb[:, :], in_=clsb_d[:, :])

            # ---- big persistent buffers ----
            xt = [bigpool.tile([128, NTOK], bf16, tag=f"xt{k}", name=f"xt{k}")
                  for k in range(3)]
            # Zb as 4 block-tiles (128 steps each) so the recurrence's bank
            # injections only depend on the projection banks of their own
            # block — rec can start after 1/4 of proj instead of all of it.
            zbt = [bigpool.tile([128, 32 * 128], bf16, tag=f"zb{nb}",
                                name=f"zb{nb}") for nb in range(4)]
            h0f = bigpool.tile([128, NTOK], bf16)
            h0b = bigpool.tile([128, NTOK], bf16)
            h1f = bigpool.tile([128, NTOK], bf16)
            h1b = bigpool.tile([128, NTOK], bf16)

            hz = spool.tile([128, 8], bf16)
            nc.vector.memset(hz[:, :], 0.0)

            def strided(tileap, offset, dims):
                return bass.AP(tensor=tileap.tensor, offset=tileap.offset + offset,
                               ap=[tileap.ap[0]] + dims)

            # ================= shared phase helpers =================
            def projection_banks(layer, pairs, pjp):
                """Emit the Zb banks for `pairs` = [(d, nb), ...]."""
                w = w0 if layer == 0 else w1
                bia = b0 if layer == 0 else b1
                nk = 3 if layer == 0 else 2
                for d, nb in pairs:
                    for c in range(4):
                        ps = pjp.tile([128, 512], f32, tag="pj")
                        s0 = 128 * nb
                        for k in range(nk):
                            if layer == 0:
                                src = xt[k][:, :]
                            else:
                                src = (h0f if k == 0 else h0b)[:, :]
                            if d == 0:
                                rhs = strided(src, 4 * s0,
                                              [[4, 128], [1, 4]])
                            else:
                                rhs = strided(src, 4 * (511 - s0),
                                              [[-4, 128], [1, 4]])
                            nc.tensor.matmul(
                                ps[:, :],
                                w[:, d, k, c * 128:(c + 1) * 128],
                                rhs, start=(k == 0), stop=(k == nk - 1))
                        dst = strided(zbt[nb][:, :], 8 * c + 4 * d,
                                      [[32, 128], [1, 4]])
                        nc.scalar.activation(
                            dst, ps[:, :], AF.Identity,
                            bias=bia[:, 4 * d + c:4 * d + c + 1], scale=1.0)

            def projection(layer):
                """Compute Zb for `layer` from its inputs (XT or H0)."""
                with tc.tile_pool(name=f"pj{layer}", bufs=4, space="PSUM") as pjp:
                    projection_banks(
                        layer, [(d, nb) for nb in range(4) for d in range(2)],
                        pjp)

            # ========== Phase A: gather + transpose, proj0 interleaved ======
            # After x-block m (gather tiles 4m..4m+3) lands, the proj0 banks
            # (d=0, nb=m) and (d=1, nb=3-m) are computable — emit them right
            # there so the PE chews projection matmuls while the next block's
            # indirect DMAs stream.
            fuse0 = phases in ('B', 'C', 'full')
            with (
                tc.tile_pool(name="xrow", bufs=4) as xrow_pool,
                tc.tile_pool(name="tpps", bufs=3, space="PSUM") as tp_pool,
                tc.tile_pool(name="pj0", bufs=4, space="PSUM") as pj0p,
            ):
                for c in range(NTILE):
                    xrow = xrow_pool.tile([128, KPAD], bf16, tag="xrow")
                    nc.gpsimd.indirect_dma_start(
                        out=xrow[:, :], out_offset=None, in_=tbl[:, :],
                        in_offset=bass.IndirectOffsetOnAxis(
                            ap=idx_sb[:, c:c + 1], axis=0),
                    )
                    for k in range(3):
                        pst = tp_pool.tile([128, 128], bf16, tag="tp")
                        nc.tensor.transpose(
                            out=pst[:, :], in_=xrow[:, k * 128:(k + 1) * 128],
                            identity=ident[:, :])
                        nc.vector.tensor_copy(
                            xt[k][:, c * 128:(c + 1) * 128], pst[:, :])
                    if fuse0 and c % 4 == 3:
                        m = c // 4
                        projection_banks(0, [(0, m), (1, 3 - m)], pj0p)

            def recurrence(layer):
                r = r0 if layer == 0 else r1
                Hf = h0f if layer == 0 else h1f
                Hb = h0b if layer == 0 else h1b
                with (
                    tc.tile_pool(name=f"rc{layer}", bufs=4 if variant == 0 else 6,
                                 space="PSUM") as rcp,
                    tc.tile_pool(name=f"gt{layer}", bufs=4 if variant == 0 else 8) as gtp,
                    tc.tile_pool(name=f"tm{layer}", bufs=3 if variant == 0 else 8) as tmp,
                ):
                    c_state = spool.tile([128, 8], f32, tag=f"c{layer}")
                    nc.vector.memset(c_state[:, :], 0.0)
                    ps = None
                    prev_ht = None
                    for s in range(T):
                        sb = s % 16
                        if sb == 0:
                            ps = rcp.tile([128, 512], f32, tag="bank")
                            bk = s // 16
                            nc.tensor.matmul(
                                ps[:, :], ident[:, :],
                                zbt[bk // 8][:, 512 * (bk % 8):512 * (bk % 8) + 512],
                                start=True, stop=False, skip_group_check=True)
                        for d in range(2):
                            if s == 0:
                                hprev = hz[:, 4 * d:4 * d + 4]
                            elif variant >= 4 and prev_ht is not None:
                                hprev = prev_ht[:, 4 * d:4 * d + 4]
                            elif d == 0:
                                hprev = Hf[:, 4 * (s - 1):4 * (s - 1) + 4]
                            else:
                                hprev = Hb[:, 4 * (512 - s):4 * (512 - s) + 4]
                            for c in range(4):
                                nc.tensor.matmul(
                                    ps[:, 32 * sb + 8 * c + 4 * d:
                                       32 * sb + 8 * c + 4 * d + 4],
                                    r[:, d, c * 128:(c + 1) * 128],
                                    hprev, start=False, stop=False,
                                    skip_group_check=True)
                        sg = gtp.tile([128, 32], f32, tag="sg")
                        nc.scalar.activation(
                            sg[:, :], ps[:, 32 * sb:32 * sb + 32], AF.Sigmoid)
                        # gate blocks are contiguous: col = 8c + 4d + e
                        i_ap = sg[:, 0:8]
                        f_ap = sg[:, 8:16]
                        g_ap = sg[:, 16:24]
                        # u = i*g' ; w = 2u - i ; v = f*c ; c = v + w
                        if variant >= 3:
                            # i*(2g'-1) = 2*i*(g'-0.5): one fused op, then the
                            # *2 folds into the final accumulate.
                            w_t = tmp.tile([128, 8], f32, tag="w")
                            nc.vector.scalar_tensor_tensor(
                                out=w_t[:, :], in0=g_ap, scalar=0.5, in1=i_ap,
                                op0=ALU.subtract, op1=ALU.mult)
                        else:
                            ueng = nc.gpsimd if variant >= 2 else nc.vector
                            u = tmp.tile([128, 8], f32, tag="u")
                            ueng.tensor_tensor(
                                out=u[:, :], in0=i_ap, in1=g_ap, op=ALU.mult)
                            w_t = tmp.tile([128, 8], f32, tag="w")
                            ueng.scalar_tensor_tensor(
                                out=w_t[:, :], in0=u[:, :], scalar=2.0, in1=i_ap,
                                op0=ALU.mult, op1=ALU.subtract)
                        v = tmp.tile([128, 8], f32, tag="v")
                        # variant 5: f*c on GpSimd, in parallel with w on DVE
                        veng = nc.gpsimd if variant >= 5 else nc.vector
                        veng.tensor_tensor(
                            out=v[:, :], in0=f_ap, in1=c_state[:, :], op=ALU.mult)
                        masked = [d for d in range(2) if (d, s) in mask_idx]
                        if not masked:
                            if variant >= 3:
                                nc.vector.scalar_tensor_tensor(
                                    out=c_state[:, :], in0=w_t[:, :], scalar=2.0,
                                    in1=v[:, :], op0=ALU.mult, op1=ALU.add)
                            else:
                                nc.vector.tensor_tensor(
                                    out=c_state[:, :], in0=v[:, :], in1=w_t[:, :],
                                    op=ALU.add)
                            th = tmp.tile([128, 8], f32, tag="th")
                            nc.scalar.activation(th[:, :], c_state[:, :], AF.Tanh)
                            if variant >= 4:
                                o_ap = sg[:, 24:32]
                                ht = tmp.tile([128, 8], bf16, tag="ht")
                                nc.vector.tensor_tensor(
                                    out=ht[:, :], in0=o_ap, in1=th[:, :],
                                    op=ALU.mult)
                                nc.vector.tensor_copy(
                                    Hf[:, 4 * s:4 * s + 4], ht[:, 0:4])
                                nc.vector.tensor_copy(
                                    Hb[:, 4 * (511 - s):4 * (511 - s) + 4],
                                    ht[:, 4:8])
                                prev_ht = ht
                            else:
                                nc.vector.tensor_tensor(
                                    out=Hf[:, 4 * s:4 * s + 4], in0=sg[:, 24:28],
                                    in1=th[:, 0:4], op=ALU.mult)
                                nc.vector.tensor_tensor(
                                    out=Hb[:, 4 * (511 - s):4 * (511 - s) + 4],
                                    in0=sg[:, 28:32], in1=th[:, 4:8], op=ALU.mult)
                        else:
                            cc = tmp.tile([128, 8], f32, tag="cc")
                            if variant >= 3:
                                nc.vector.scalar_tensor_tensor(
                                    out=cc[:, :], in0=w_t[:, :], scalar=2.0,
                                    in1=v[:, :], op0=ALU.mult, op1=ALU.add)
                            else:
                                nc.vector.tensor_tensor(
                                    out=cc[:, :], in0=v[:, :], in1=w_t[:, :], op=ALU.add)
                            # c lerp: cc_d = c_old + m*(cc_d - c_old)
                            for d in masked:
                                mi = mask_idx[(d, s)]
                                mcol = msk[:, 4 * mi:4 * mi + 4]
                                dd = tmp.tile([128, 4], f32, tag="dd")
                                nc.vector.tensor_tensor(
                                    out=dd[:, :], in0=cc[:, 4 * d:4 * d + 4],
                                    in1=c_state[:, 4 * d:4 * d + 4], op=ALU.subtract)
                                nc.vector.tensor_tensor(
                                    out=dd[:, :], in0=dd[:, :], in1=mcol, op=ALU.mult)
                                nc.vector.tensor_tensor(
                                    out=cc[:, 4 * d:4 * d + 4], in0=dd[:, :],
                                    in1=c_state[:, 4 * d:4 * d + 4], op=ALU.add)
                            nc.vector.tensor_copy(c_state[:, :], cc[:, :])
                            th = tmp.tile([128, 8], f32, tag="th")
                            nc.scalar.activation(th[:, :], c_state[:, :], AF.Tanh)
                            for d in range(2):
                                o_sl = sg[:, 24 + 4 * d:28 + 4 * d]
                                th_sl = th[:, 4 * d:4 * d + 4]
                                dst = (Hf[:, 4 * s:4 * s + 4] if d == 0 else
                                       Hb[:, 4 * (511 - s):4 * (511 - s) + 4])
                                if d in masked:
                                    mi = mask_idx[(d, s)]
                                    mcol = msk[:, 4 * mi:4 * mi + 4]
                                    if s == 0:
                                        hp = hz[:, 4 * d:4 * d + 4]
                                    elif d == 0:
                                        hp = Hf[:, 4 * (s - 1):4 * (s - 1) + 4]
                                    else:
                                        hp = Hb[:, 4 * (512 - s):4 * (512 - s) + 4]
                                    hn = tmp.tile([128, 4], f32, tag="hn")
                                    nc.vector.tensor_tensor(
                                        out=hn[:, :], in0=o_sl, in1=th_sl,
                                        op=ALU.mult)
                                    nc.vector.tensor_tensor(
                                        out=hn[:, :], in0=hn[:, :], in1=hp,
                                        op=ALU.subtract)
                                    nc.vector.tensor_tensor(
                                        out=hn[:, :], in0=hn[:, :], in1=mcol,
                                        op=ALU.mult)
                                    nc.vector.tensor_tensor(
                                        out=dst, in0=hn[:, :], in1=hp, op=ALU.add)
                                else:
                                    nc.vector.tensor_tensor(
                                        out=dst, in0=o_sl, in1=th_sl, op=ALU.mult)
                            prev_ht = None

            # ================= run the phases =================
            # (proj0 is emitted inside Phase A, interleaved with the gather)
            if phases in ('C', 'full'):
                recurrence(0)
            if phases == 'full':
                projection(1)
                recurrence(1)

            # ================= classifier + softmax =================
            with (
                tc.tile_pool(name="cls", bufs=4) as clp,
                tc.tile_pool(name="clps", bufs=4, space="PSUM") as clps,
            ):
                for tt in range(NTILE if phases == 'full' else 1):
                    sl = slice(128 * tt, 128 * (tt + 1))
                    i0 = clp.tile([128, 128], bf16, tag="i0")
                    nc.vector.tensor_tensor(
                        out=i0[:, :], in0=h0f[:, sl], in1=h1f[:, sl], op=ALU.add)
                    i1 = clp.tile([128, 128], bf16, tag="i1")
                    nc.vector.tensor_tensor(
                        out=i1[:, :], in0=h0b[:, sl], in1=h1b[:, sl], op=ALU.add)
                    pc = clps.tile([128, NCLS], f32, tag="pc")
                    nc.tensor.matmul(pc[:, :], i0[:, :], clsw[:, 0, :],
                                     start=True, stop=False)
                    nc.tensor.matmul(pc[:, :], i1[:, :], clsw[:, 1, :],
                                     start=False, stop=True)
                    ex = clp.tile([128, NCLS], f32, tag="ex")
                    if has_clsb:
                        nc.vector.tensor_tensor(
                            out=ex[:, :], in0=pc[:, :], in1=clsb[:, :], op=ALU.add)
                        nc.scalar.activation(ex[:, :], ex[:, :], AF.Exp)
                    else:
                        nc.scalar.activation(ex[:, :], pc[:, :], AF.Exp)
                    ssum = clp.tile([128, 1], f32, tag="ss")
                    nc.vector.tensor_reduce(
                        out=ssum[:, :], in_=ex[:, :], op=ALU.add,
                        axis=mybir.AxisListType.X)
                    rec_t = clp.tile([128, 1], f32, tag="rc")
                    nc.vector.reciprocal(rec_t[:, :], ssum[:, :])
                    sm = clp.tile([128, NCLS], f32, tag="sm")
                    nc.vector.tensor_scalar_mul(sm[:, :], ex[:, :], rec_t[:, :])
                    nc.gpsimd.dma_start(out=out_d[sl, :], in_=sm[:, :])

    return nc


# ---------------------------------------------------------------------------
def _prep_host(inputs):
    """Shard + pre-arrange all device inputs. Returns (in_maps, mask_entries,
    has_clsb)."""
    ids = np.asarray(inputs["ids"])
    emb = np.asarray(inputs["emb_table"], dtype=np.float32)

    def gate2(wk):
        w = np.array(wk, dtype=np.float32, copy=True)
        w[:, 2 * UNITS:3 * UNITS] *= 2.0
        return w

    def pad_k(w, kpad):
        out = np.zeros((kpad, G4), np.float32)
        out[:w.shape[0]] = w
        return out

    w0 = np.stack([
        pad_k(gate2(inputs["fw0_k"]), KPAD).reshape(3, 128, G4),
        pad_k(gate2(inputs["bw0_k"]), KPAD).reshape(3, 128, G4),
    ]).astype(ml_dtypes.bfloat16)
    r0 = np.stack([gate2(inputs["fw0_r"]), gate2(inputs["bw0_r"])]
                  ).astype(ml_dtypes.bfloat16)
    w1 = np.stack([
        gate2(inputs["fw1_k"]).reshape(2, 128, G4),
        gate2(inputs["bw1_k"]).reshape(2, 128, G4),
    ]).astype(ml_dtypes.bfloat16)
    r1 = np.stack([gate2(inputs["fw1_r"]), gate2(inputs["bw1_r"])]
                  ).astype(ml_dtypes.bfloat16)

    def bias_tile(bf, bb):
        out = np.zeros((128, 8), np.float32)
        for d, b in enumerate((bf, bb)):
            b = np.array(b, dtype=np.float32, copy=True)
            b[2 * UNITS:3 * UNITS] *= 2.0
            out[:, 4 * d:4 * d + 4] = b.reshape(4, 128).T
        return out

    b0 = bias_tile(inputs["fw0_b"], inputs["bw0_b"])
    b1 = bias_tile(inputs["fw1_b"], inputs["bw1_b"])
    clsw = np.asarray(inputs["cls_w"], np.float32).reshape(2, 128, NCLS).astype(
        ml_dtypes.bfloat16)
    clsb_np = np.asarray(inputs["cls_b"], np.float32)
    has_clsb = bool(np.any(clsb_np != 0))
    ident = np.eye(128, dtype=ml_dtypes.bfloat16)

    mask_entry_set = set()
    per_core = []
    for c in range(NCORES):
        ids_c = ids[BL * c:BL * (c + 1)].astype(np.int64)      # [BL, T]
        ids_tm = ids_c.T.reshape(-1)                           # j = t*BL + e
        uniq, inv = np.unique(ids_tm, return_inverse=True)
        tblp = np.zeros((NU, KPAD), ml_dtypes.bfloat16)
        tblp[:len(uniq), :EMB] = emb[uniq].astype(ml_dtypes.bfloat16)
        idx_np = inv.astype(np.int32).reshape(NTILE, 128).T.copy()
        mask_c = (ids_c != 0)
        for e, t in zip(*np.nonzero(~mask_c)):
            mask_entry_set.add((0, int(t)))          # fwd step s = t
            mask_entry_set.add((1, int(511 - t)))    # bwd step s = 511 - t
        per_core.append((tblp, idx_np, mask_c))

    mask_entries = tuple(sorted(mask_entry_set))
    nmask = max(1, len(mask_entries))

    in_maps = []
    for c in range(NCORES):
        tblp, idx_np, mask_c = per_core[c]
        msk = np.ones((128, 4 * nmask), np.float32)
        for mi, (d, s) in enumerate(mask_entries):
            t = s if d == 0 else 511 - s
            msk[:, 4 * mi:4 * mi + 4] = mask_c[:, t].astype(np.float32)[None, :]
        m = dict(tbl=tblp, idx=idx_np, ident=ident, w0=w0, r0=r0, w1=w1, r1=r1,
                 b0=b0, b1=b1, clsw=clsw, msk=msk)
        if has_clsb:
            m["clsb"] = np.broadcast_to(
                clsb_np.astype(np.float32), (128, NCLS)).copy()
        in_maps.append(m)
    return in_maps, mask_entries, has_clsb


# ---------------------------------------------------------------------------
def kernel(**inputs):
    from concourse.bass_utils import run_bass_kernel_spmd

    in_maps, mask_entries, has_clsb = _prep_host(inputs)

    key = (mask_entries, has_clsb, VARIANT)
    if key not in _prog_cache:
        _prog_cache[key] = _build_program(mask_entries, has_clsb,
                                          variant=VARIANT)
    nc = _prog_cache[key]

    res = run_bass_kernel_spmd(nc, in_maps, core_ids=list(range(NCORES)))

    out = np.empty((B, T, NCLS), np.float32)
    for c in range(NCORES):
        oc = res.results[c]["out"].reshape(T, BL, NCLS)
        out[BL * c:BL * (c + 1)] = oc.transpose(1, 0, 2)
    return out



# revision 26
# speedup vs baseline: 14.5087x; 1.0028x over previous
"""Trainium2 Bass kernel for a 2-layer BiLSTM text tagger.

Model (see reference): embedding gather -> BiLSTM(128) -> BiLSTM(128) with
residual -> dense(279) -> softmax. mask_zero=True semantics (state + output
carry-through at masked steps).

Sharding: data-parallel over batch, 4 examples per core on 8 cores. Each core
runs the full network for its slice; no collectives.

Device layout (per core, "layout B" — feature/gate dim on partitions, batch in
the free dim):
  XT[k]  [128, 2048] bf16  - gathered embeddings, transposed; feature = 128k+p,
                             token col j = 4t+e (t-major, e = local example)
  Zb     [128, 16384] bf16 - input projections in PSUM-bank order:
                             col = 32s + 8c + 4d + e (s step, c gate chunk
                             i/f/g/o, d dir, e example) — gate blocks
                             contiguous. g-chunk pre-scaled by 2 so one
                             Sigmoid over all 32 cols computes i,f,o sigmoids
                             and sigma(2 z_g) (tanh via 2*sig(2x)-1).
  H*     [128, 2048] bf16  - hidden states, col = 4t + e
  Recurrence step: one identity-matmul injects 16 steps of Z into a PSUM bank
  (start=True), then per step 8 accumulating matmuls add h @ Wr per
  (dir, gate-chunk); Sigmoid reads the 32-col slice; DVE computes the cell
  update with a fused scalar_tensor_tensor for the tanh fix-up.
"""

import json

import ml_dtypes
import numpy as np

# ---------------------------------------------------------------------------
# problem constants (hardcoded per the contract)
B, T = 32, 512
EMB, UNITS, NCLS = 300, 128, 279
VOCAB = 100000
NCORES = 8
BL = B // NCORES          # 4 examples / core
NTOK = BL * T             # 2048 tokens / core
G4 = 4 * UNITS            # 512
KPAD = 384                # padded embedding dim (3 x 128)
NU = 2048                 # compact table rows (fixed shape across cores)
NTILE = NTOK // 128       # 16 token tiles
VARIANT = 4               # recurrence micro-schedule variant

_prog_cache = {}


# ---------------------------------------------------------------------------
def _apply_bir_wait_split(bass_mod):
    """This container's walrus rejects >1 sync-wait per instruction. Split
    extras onto inserted EventSemaphore instructions (same engine, in order).
    """
    if getattr(bass_mod.Bass, "_wait_split_applied", False):
        return
    orig = bass_mod.Bass.to_json_bytes
    ctr = [0]

    def fix_list(lst):
        out, changed = [], False
        for ins in lst:
            si = ins.get("sync_info") if isinstance(ins, dict) else None
            if not si:
                out.append(ins)
                continue
            waits = si.get("on_wait") or []
            upds = si.get("on_update") or []
            if len(waits) > 1:
                for w in waits[1:]:
                    ctr[0] += 1
                    out.append({
                        "debug": ins.get("debug", 0), "engine": ins["engine"],
                        "ins": [], "name": f"I-waitfix-{ctr[0]}",
                        "opcode": "EventSemaphore", "outs": [],
                        "sync_info": {"on_update": [], "on_wait": [w]},
                    })
                si["on_wait"] = waits[:1]
                changed = True
            out.append(ins)
            if len(upds) > 1:
                for u in upds[1:]:
                    ctr[0] += 1
                    out.append({
                        "debug": ins.get("debug", 0), "engine": ins["engine"],
                        "ins": [], "name": f"I-updfix-{ctr[0]}",
                        "opcode": "EventSemaphore", "outs": [],
                        "sync_info": {"on_update": [u], "on_wait": []},
                    })
                si["on_update"] = upds[:1]
                changed = True
        return out, changed

    def walk(o):
        if isinstance(o, dict):
            for k, v in o.items():
                if (isinstance(v, list) and v
                        and all(isinstance(e, dict) and "opcode" in e for e in v)):
                    fixed, changed = fix_list(v)
                    if changed:
                        o[k] = fixed
                    for e in o[k]:
                        walk(e)
                else:
                    walk(v)
        elif isinstance(o, list):
            for v in o:
                walk(v)

    def to_json_bytes_fixed(self):
        d = json.loads(orig(self))
        walk(d)
        return json.dumps(d).encode()

    bass_mod.Bass.to_json_bytes = to_json_bytes_fixed
    bass_mod.Bass._wait_split_applied = True


# ---------------------------------------------------------------------------
def _build_program(mask_entries, has_clsb, phases='full', variant=4):
    """Build the Bass program (shared by all 8 cores).

    mask_entries: sorted tuple of (d, s) recurrence slots that need the
    data-driven carry-through lerp (d: 0 fwd / 1 bwd, s: step index).
    """
    import concourse.bass as bass
    import concourse.mybir as mybir
    import concourse.tile as tile

    _apply_bir_wait_split(bass)

    bf16 = mybir.dt.bfloat16
    f32 = mybir.dt.float32
    i32 = mybir.dt.int32
    AF = mybir.ActivationFunctionType
    ALU = mybir.AluOpType

    nc = bass.Bass()

    # ---- DRAM I/O ----
    tbl = nc.dram_tensor("tbl", [NU, KPAD], bf16, kind="ExternalInput")
    idx = nc.dram_tensor("idx", [128, NTILE], i32, kind="ExternalInput")
    ident_d = nc.dram_tensor("ident", [128, 128], bf16, kind="ExternalInput")
    w0_d = nc.dram_tensor("w0", [2, 3, 128, G4], bf16, kind="ExternalInput")
    r0_d = nc.dram_tensor("r0", [2, 128, G4], bf16, kind="ExternalInput")
    w1_d = nc.dram_tensor("w1", [2, 2, 128, G4], bf16, kind="ExternalInput")
    r1_d = nc.dram_tensor("r1", [2, 128, G4], bf16, kind="ExternalInput")
    b0_d = nc.dram_tensor("b0", [128, 8], f32, kind="ExternalInput")
    b1_d = nc.dram_tensor("b1", [128, 8], f32, kind="ExternalInput")
    clsw_d = nc.dram_tensor("clsw", [2, 128, NCLS], bf16, kind="ExternalInput")
    nmask = max(1, len(mask_entries))
    msk_d = nc.dram_tensor("msk", [128, 4 * nmask], f32, kind="ExternalInput")
    clsb_d = None
    if has_clsb:
        clsb_d = nc.dram_tensor("clsb", [128, NCLS], f32, kind="ExternalInput")
    out_d = nc.dram_tensor("out", [NTOK, NCLS], f32, kind="ExternalOutput")

    mask_idx = {ds: i for i, ds in enumerate(mask_entries)}

    with tile.TileContext(nc) as tc:
        with (
            tc.tile_pool(name="const", bufs=1) as cpool,
            tc.tile_pool(name="big", bufs=1) as bigpool,
            tc.tile_pool(name="state", bufs=1) as spool,
        ):
            # ---- constants to SBUF ----
            # idx/ident gate the gather phase: keep them first on the gpsimd
            # queue. Everything else (needed only from proj0 onward) goes on
            # the idle sync-engine queue so the indirect gather DMAs are not
            # stuck behind ~7us of constant-DMA issue.
            idx_sb = cpool.tile([128, NTILE], i32)
            nc.gpsimd.dma_start(out=idx_sb[:, :], in_=idx[:, :])
            ident = cpool.tile([128, 128], bf16)
            nc.gpsimd.dma_start(out=ident[:, :], in_=ident_d[:, :])
            w0 = cpool.tile([128, 2, 3, G4], bf16)
            nc.sync.dma_start(
                out=w0[:, :, :, :], in_=w0_d.rearrange("d k p g -> p d k g"))
            r0 = cpool.tile([128, 2, G4], bf16)
            nc.sync.dma_start(out=r0[:, :, :], in_=r0_d.rearrange("d p g -> p d g"))
            w1 = cpool.tile([128, 2, 2, G4], bf16)
            nc.sync.dma_start(
                out=w1[:, :, :, :], in_=w1_d.rearrange("d k p g -> p d k g"))
            r1 = cpool.tile([128, 2, G4], bf16)
            nc.sync.dma_start(out=r1[:, :, :], in_=r1_d.rearrange("d p g -> p d g"))
            b0 = cpool.tile([128, 8], f32)
            nc.sync.dma_start(out=b0[:, :], in_=b0_d[:, :])
            b1 = cpool.tile([128, 8], f32)
            nc.sync.dma_start(out=b1[:, :], in_=b1_d[:, :])
            clsw = cpool.tile([128, 2, NCLS], bf16)
            nc.sync.dma_start(out=clsw[:, :, :], in_=clsw_d.rearrange("k p n -> p k n"))
            msk = cpool.tile([128, 4 * nmask], f32)
            nc.sync.dma_start(out=msk[:, :], in_=msk_d[:, :])
            clsb = None
            if has_clsb:
                clsb = cpool.tile([128, NCLS], f32)
                nc.sync.dma_start(out=clsb[:, :], in_=clsb_d[:, :])

            # ---- big persistent buffers ----
            xt = [bigpool.tile([128, NTOK], bf16, tag=f"xt{k}", name=f"xt{k}")
                  for k in range(3)]
            # Zb as 4 block-tiles (128 steps each) so the recurrence's bank
            # injections only depend on the projection banks of their own
            # block — rec can start after 1/4 of proj instead of all of it.
            zbt = [bigpool.tile([128, 32 * 128], bf16, tag=f"zb{nb}",
                                name=f"zb{nb}") for nb in range(4)]
            # h0 split into 4 token-block tiles so proj1 banks that only need
            # the middle blocks (complete by rec0 step 383) can be emitted
            # into rec0's tail; h1 stays whole (classifier needs it all).
            h0f = [bigpool.tile([128, 512], bf16, tag=f"h0f{b}",
                                name=f"h0f{b}") for b in range(4)]
            h0b = [bigpool.tile([128, 512], bf16, tag=f"h0b{b}",
                               name=f"h0b{b}") for b in range(4)]
            h1f = bigpool.tile([128, NTOK], bf16)
            h1b = bigpool.tile([128, NTOK], bf16)

            def h_ap(H, col, width=4):
                """Slice a (possibly block-split) H buffer at `col`."""
                if isinstance(H, list):
                    b = col // 512
                    return H[b][:, col - 512 * b:col - 512 * b + width]
                return H[:, col:col + width]

            hz = spool.tile([128, 8], bf16)
            nc.vector.memset(hz[:, :], 0.0)

            def strided(tileap, offset, dims):
                return bass.AP(tensor=tileap.tensor, offset=tileap.offset + offset,
                               ap=[tileap.ap[0]] + dims)

            # ================= shared phase helpers =================
            def projection_bank_chunk(layer, d, nb, c, pjp):
                """Emit one Zb bank (gate chunk c of (d, nb))."""
                w = w0 if layer == 0 else w1
                bia = b0 if layer == 0 else b1
                nk = 3 if layer == 0 else 2
                ps = pjp.tile([128, 512], f32, tag="pj")
                s0 = 128 * nb
                for k in range(nk):
                    if layer == 0:
                        src = xt[k][:, :]
                        if d == 0:
                            rhs = strided(src, 4 * s0, [[4, 128], [1, 4]])
                        else:
                            rhs = strided(src, 4 * (511 - s0),
                                          [[-4, 128], [1, 4]])
                    else:
                        # token block nb (d=0) / 3-nb (d=1); offsets within
                        # the 512-col block tile.
                        blk = nb if d == 0 else 3 - nb
                        src = (h0f if k == 0 else h0b)[blk][:, :]
                        if d == 0:
                            rhs = strided(src, 0, [[4, 128], [1, 4]])
                        else:
                            rhs = strided(src, 508, [[-4, 128], [1, 4]])
                    nc.tensor.matmul(
                        ps[:, :], w[:, d, k, c * 128:(c + 1) * 128],
                        rhs, start=(k == 0), stop=(k == nk - 1))
                dst = strided(zbt[nb][:, :], 8 * c + 4 * d,
                              [[32, 128], [1, 4]])
                nc.scalar.activation(
                    dst, ps[:, :], AF.Identity,
                    bias=bia[:, 4 * d + c:4 * d + c + 1], scale=1.0)

            def projection_banks(layer, pairs, pjp):
                """Emit the Zb banks for `pairs` = [(d, nb), ...]."""
                for d, nb in pairs:
                    for c in range(4):
                        projection_bank_chunk(layer, d, nb, c, pjp)

            # ========== Phase A: gather + transpose, proj0 interleaved ======
            # After x-block m (gather tiles 4m..4m+3) lands, the proj0 banks
            # (d=0, nb=m) and (d=1, nb=3-m) are computable — emit them right
            # there so the PE chews projection matmuls while the next block's
            # indirect DMAs stream.
            fuse0 = phases in ('B', 'C', 'full')
            with (
                tc.tile_pool(name="xrow", bufs=4) as xrow_pool,
                tc.tile_pool(name="tpps", bufs=3, space="PSUM") as tp_pool,
                tc.tile_pool(name="pj0", bufs=4, space="PSUM") as pj0p,
            ):
                for c in range(NTILE):
                    xrow = xrow_pool.tile([128, KPAD], bf16, tag="xrow")
                    nc.gpsimd.indirect_dma_start(
                        out=xrow[:, :], out_offset=None, in_=tbl[:, :],
                        in_offset=bass.IndirectOffsetOnAxis(
                            ap=idx_sb[:, c:c + 1], axis=0),
                    )
                    for k in range(3):
                        pst = tp_pool.tile([128, 128], bf16, tag="tp")
                        nc.tensor.transpose(
                            out=pst[:, :], in_=xrow[:, k * 128:(k + 1) * 128],
                            identity=ident[:, :])
                        nc.vector.tensor_copy(
                            xt[k][:, c * 128:(c + 1) * 128], pst[:, :])
                    if fuse0 and c % 4 == 3:
                        m = c // 4
                        projection_banks(0, [(0, m), (1, 3 - m)], pj0p)

            def recurrence(layer):
                r = r0 if layer == 0 else r1
                Hf = h0f if layer == 0 else h1f
                Hb = h0b if layer == 0 else h1b
                # proj1 banks whose h0 blocks are complete by step 383 get
                # emitted into rec0's tail (one gate-chunk every 7 steps);
                # their matmuls fit the PE's per-step idle window.
                inter = {}
                if layer == 0 and phases == 'full':
                    units = [(d, nb, cc)
                             for (d, nb) in ((0, 1), (0, 2), (1, 1), (1, 2))
                             for cc in range(4)]
                    for j, u in enumerate(units):
                        inter[388 + 7 * j] = u
                with (
                    tc.tile_pool(name=f"rc{layer}", bufs=4 if variant == 0 else 6,
                                 space="PSUM") as rcp,
                    tc.tile_pool(name=f"pji{layer}", bufs=2, space="PSUM") as pjip,
                    tc.tile_pool(name=f"gt{layer}", bufs=4 if variant == 0 else 8) as gtp,
                    tc.tile_pool(name=f"tm{layer}", bufs=3 if variant == 0 else 8) as tmp,
                ):
                    c_state = spool.tile([128, 8], f32, tag=f"c{layer}")
                    nc.vector.memset(c_state[:, :], 0.0)
                    ps = None
                    prev_ht = None
                    for s in range(T):
                        sb = s % 16
                        if sb == 0:
                            ps = rcp.tile([128, 512], f32, tag="bank")
                            bk = s // 16
                            nc.tensor.matmul(
                                ps[:, :], ident[:, :],
                                zbt[bk // 8][:, 512 * (bk % 8):512 * (bk % 8) + 512],
                                start=True, stop=False, skip_group_check=True)
                        for d in range(2):
                            if s == 0:
                                hprev = hz[:, 4 * d:4 * d + 4]
                            elif variant >= 4 and prev_ht is not None:
                                hprev = prev_ht[:, 4 * d:4 * d + 4]
                            elif d == 0:
                                hprev = h_ap(Hf, 4 * (s - 1))
                            else:
                                hprev = h_ap(Hb, 4 * (512 - s))
                            for c in range(4):
                                nc.tensor.matmul(
                                    ps[:, 32 * sb + 8 * c + 4 * d:
                                       32 * sb + 8 * c + 4 * d + 4],
                                    r[:, d, c * 128:(c + 1) * 128],
                                    hprev, start=False, stop=False,
                                    skip_group_check=True)
                        sg = gtp.tile([128, 32], f32, tag="sg")
                        nc.scalar.activation(
                            sg[:, :], ps[:, 32 * sb:32 * sb + 32], AF.Sigmoid)
                        # gate blocks are contiguous: col = 8c + 4d + e
                        i_ap = sg[:, 0:8]
                        f_ap = sg[:, 8:16]
                        g_ap = sg[:, 16:24]
                        # u = i*g' ; w = 2u - i ; v = f*c ; c = v + w
                        if variant >= 3:
                            # i*(2g'-1) = 2*i*(g'-0.5): one fused op, then the
                            # *2 folds into the final accumulate.
                            w_t = tmp.tile([128, 8], f32, tag="w")
                            nc.vector.scalar_tensor_tensor(
                                out=w_t[:, :], in0=g_ap, scalar=0.5, in1=i_ap,
                                op0=ALU.subtract, op1=ALU.mult)
                        else:
                            ueng = nc.gpsimd if variant >= 2 else nc.vector
                            u = tmp.tile([128, 8], f32, tag="u")
                            ueng.tensor_tensor(
                                out=u[:, :], in0=i_ap, in1=g_ap, op=ALU.mult)
                            w_t = tmp.tile([128, 8], f32, tag="w")
                            ueng.scalar_tensor_tensor(
                                out=w_t[:, :], in0=u[:, :], scalar=2.0, in1=i_ap,
                                op0=ALU.mult, op1=ALU.subtract)
                        v = tmp.tile([128, 8], f32, tag="v")
                        # variant 5: f*c on GpSimd, in parallel with w on DVE
                        veng = nc.gpsimd if variant >= 5 else nc.vector
                        veng.tensor_tensor(
                            out=v[:, :], in0=f_ap, in1=c_state[:, :], op=ALU.mult)
                        masked = [d for d in range(2) if (d, s) in mask_idx]
                        if not masked:
                            if variant >= 3:
                                nc.vector.scalar_tensor_tensor(
                                    out=c_state[:, :], in0=w_t[:, :], scalar=2.0,
                                    in1=v[:, :], op0=ALU.mult, op1=ALU.add)
                            else:
                                nc.vector.tensor_tensor(
                                    out=c_state[:, :], in0=v[:, :], in1=w_t[:, :],
                                    op=ALU.add)
                            th = tmp.tile([128, 8], f32, tag="th")
                            nc.scalar.activation(th[:, :], c_state[:, :], AF.Tanh)
                            if variant >= 4:
                                o_ap = sg[:, 24:32]
                                ht = tmp.tile([128, 8], bf16, tag="ht")
                                nc.vector.tensor_tensor(
                                    out=ht[:, :], in0=o_ap, in1=th[:, :],
                                    op=ALU.mult)
                                nc.vector.tensor_copy(
                                    h_ap(Hf, 4 * s), ht[:, 0:4])
                                nc.vector.tensor_copy(
                                    h_ap(Hb, 4 * (511 - s)), ht[:, 4:8])
                                prev_ht = ht
                            else:
                                nc.vector.tensor_tensor(
                                    out=h_ap(Hf, 4 * s), in0=sg[:, 24:28],
                                    in1=th[:, 0:4], op=ALU.mult)
                                nc.vector.tensor_tensor(
                                    out=h_ap(Hb, 4 * (511 - s)),
                                    in0=sg[:, 28:32], in1=th[:, 4:8], op=ALU.mult)
                        else:
                            cc = tmp.tile([128, 8], f32, tag="cc")
                            if variant >= 3:
                                nc.vector.scalar_tensor_tensor(
                                    out=cc[:, :], in0=w_t[:, :], scalar=2.0,
                                    in1=v[:, :], op0=ALU.mult, op1=ALU.add)
                            else:
                                nc.vector.tensor_tensor(
                                    out=cc[:, :], in0=v[:, :], in1=w_t[:, :], op=ALU.add)
                            # c lerp: cc_d = c_old + m*(cc_d - c_old)
                            for d in masked:
                                mi = mask_idx[(d, s)]
                                mcol = msk[:, 4 * mi:4 * mi + 4]
                                dd = tmp.tile([128, 4], f32, tag="dd")
                                nc.vector.tensor_tensor(
                                    out=dd[:, :], in0=cc[:, 4 * d:4 * d + 4],
                                    in1=c_state[:, 4 * d:4 * d + 4], op=ALU.subtract)
                                nc.vector.tensor_tensor(
                                    out=dd[:, :], in0=dd[:, :], in1=mcol, op=ALU.mult)
                                nc.vector.tensor_tensor(
                                    out=cc[:, 4 * d:4 * d + 4], in0=dd[:, :],
                                    in1=c_state[:, 4 * d:4 * d + 4], op=ALU.add)
                            nc.vector.tensor_copy(c_state[:, :], cc[:, :])
                            th = tmp.tile([128, 8], f32, tag="th")
                            nc.scalar.activation(th[:, :], c_state[:, :], AF.Tanh)
                            for d in range(2):
                                o_sl = sg[:, 24 + 4 * d:28 + 4 * d]
                                th_sl = th[:, 4 * d:4 * d + 4]
                                dst = (h_ap(Hf, 4 * s) if d == 0 else
                                       h_ap(Hb, 4 * (511 - s)))
                                if d in masked:
                                    mi = mask_idx[(d, s)]
                                    mcol = msk[:, 4 * mi:4 * mi + 4]
                                    if s == 0:
                                        hp = hz[:, 4 * d:4 * d + 4]
                                    elif d == 0:
                                        hp = h_ap(Hf, 4 * (s - 1))
                                    else:
                                        hp = h_ap(Hb, 4 * (512 - s))
                                    hn = tmp.tile([128, 4], f32, tag="hn")
                                    nc.vector.tensor_tensor(
                                        out=hn[:, :], in0=o_sl, in1=th_sl,
                                        op=ALU.mult)
                                    nc.vector.tensor_tensor(
                                        out=hn[:, :], in0=hn[:, :], in1=hp,
                                        op=ALU.subtract)
                                    nc.vector.tensor_tensor(
                                        out=hn[:, :], in0=hn[:, :], in1=mcol,
                                        op=ALU.mult)
                                    nc.vector.tensor_tensor(
                                        out=dst, in0=hn[:, :], in1=hp, op=ALU.add)
                                else:
                                    nc.vector.tensor_tensor(
                                        out=dst, in0=o_sl, in1=th_sl, op=ALU.mult)
                            prev_ht = None
                        u = inter.get(s)
                        if u is not None:
                            projection_bank_chunk(1, u[0], u[1], u[2], pjip)

            # ================= run the phases =================
            # (proj0 is emitted inside Phase A, interleaved with the gather;
            # proj1's middle-block banks are emitted inside rec0's tail —
            # only the zbt[0]/zbt[3] banks remain between the layers.)
            if phases in ('C', 'full'):
                recurrence(0)
            if phases == 'full':
                with tc.tile_pool(name="pj1", bufs=4, space="PSUM") as pj1p:
                    projection_banks(
                        1, [(0, 0), (1, 0), (0, 3), (1, 3)], pj1p)
                recurrence(1)

            # ================= classifier + softmax =================
            with (
                tc.tile_pool(name="cls", bufs=4) as clp,
                tc.tile_pool(name="clps", bufs=4, space="PSUM") as clps,
            ):
                for tt in range(NTILE if phases == 'full' else 1):
                    sl = slice(128 * tt, 128 * (tt + 1))
                    i0 = clp.tile([128, 128], bf16, tag="i0")
                    nc.vector.tensor_tensor(
                        out=i0[:, :], in0=h_ap(h0f, 128 * tt, 128),
                        in1=h1f[:, sl], op=ALU.add)
                    i1 = clp.tile([128, 128], bf16, tag="i1")
                    nc.vector.tensor_tensor(
                        out=i1[:, :], in0=h_ap(h0b, 128 * tt, 128),
                        in1=h1b[:, sl], op=ALU.add)
                    pc = clps.tile([128, NCLS], f32, tag="pc")
                    nc.tensor.matmul(pc[:, :], i0[:, :], clsw[:, 0, :],
                                     start=True, stop=False)
                    nc.tensor.matmul(pc[:, :], i1[:, :], clsw[:, 1, :],
                                     start=False, stop=True)
                    ex = clp.tile([128, NCLS], f32, tag="ex")
                    if has_clsb:
                        nc.vector.tensor_tensor(
                            out=ex[:, :], in0=pc[:, :], in1=clsb[:, :], op=ALU.add)
                        nc.scalar.activation(ex[:, :], ex[:, :], AF.Exp)
                    else:
                        nc.scalar.activation(ex[:, :], pc[:, :], AF.Exp)
                    ssum = clp.tile([128, 1], f32, tag="ss")
                    nc.vector.tensor_reduce(
                        out=ssum[:, :], in_=ex[:, :], op=ALU.add,
                        axis=mybir.AxisListType.X)
                    rec_t = clp.tile([128, 1], f32, tag="rc")
                    nc.vector.reciprocal(rec_t[:, :], ssum[:, :])
                    sm = clp.tile([128, NCLS], f32, tag="sm")
                    nc.vector.tensor_scalar_mul(sm[:, :], ex[:, :], rec_t[:, :])
                    nc.gpsimd.dma_start(out=out_d[sl, :], in_=sm[:, :])

    return nc


# ---------------------------------------------------------------------------
def _prep_host(inputs):
    """Shard + pre-arrange all device inputs. Returns (in_maps, mask_entries,
    has_clsb)."""
    ids = np.asarray(inputs["ids"])
    emb = np.asarray(inputs["emb_table"], dtype=np.float32)

    def gate2(wk):
        w = np.array(wk, dtype=np.float32, copy=True)
        w[:, 2 * UNITS:3 * UNITS] *= 2.0
        return w

    def pad_k(w, kpad):
        out = np.zeros((kpad, G4), np.float32)
        out[:w.shape[0]] = w
        return out

    w0 = np.stack([
        pad_k(gate2(inputs["fw0_k"]), KPAD).reshape(3, 128, G4),
        pad_k(gate2(inputs["bw0_k"]), KPAD).reshape(3, 128, G4),
    ]).astype(ml_dtypes.bfloat16)
    r0 = np.stack([gate2(inputs["fw0_r"]), gate2(inputs["bw0_r"])]
                  ).astype(ml_dtypes.bfloat16)
    w1 = np.stack([
        gate2(inputs["fw1_k"]).reshape(2, 128, G4),
        gate2(inputs["bw1_k"]).reshape(2, 128, G4),
    ]).astype(ml_dtypes.bfloat16)
    r1 = np.stack([gate2(inputs["fw1_r"]), gate2(inputs["bw1_r"])]
                  ).astype(ml_dtypes.bfloat16)

    def bias_tile(bf, bb):
        out = np.zeros((128, 8), np.float32)
        for d, b in enumerate((bf, bb)):
            b = np.array(b, dtype=np.float32, copy=True)
            b[2 * UNITS:3 * UNITS] *= 2.0
            out[:, 4 * d:4 * d + 4] = b.reshape(4, 128).T
        return out

    b0 = bias_tile(inputs["fw0_b"], inputs["bw0_b"])
    b1 = bias_tile(inputs["fw1_b"], inputs["bw1_b"])
    clsw = np.asarray(inputs["cls_w"], np.float32).reshape(2, 128, NCLS).astype(
        ml_dtypes.bfloat16)
    clsb_np = np.asarray(inputs["cls_b"], np.float32)
    has_clsb = bool(np.any(clsb_np != 0))
    ident = np.eye(128, dtype=ml_dtypes.bfloat16)

    mask_entry_set = set()
    per_core = []
    for c in range(NCORES):
        ids_c = ids[BL * c:BL * (c + 1)].astype(np.int64)      # [BL, T]
        ids_tm = ids_c.T.reshape(-1)                           # j = t*BL + e
        uniq, inv = np.unique(ids_tm, return_inverse=True)
        tblp = np.zeros((NU, KPAD), ml_dtypes.bfloat16)
        tblp[:len(uniq), :EMB] = emb[uniq].astype(ml_dtypes.bfloat16)
        idx_np = inv.astype(np.int32).reshape(NTILE, 128).T.copy()
        mask_c = (ids_c != 0)
        for e, t in zip(*np.nonzero(~mask_c)):
            mask_entry_set.add((0, int(t)))          # fwd step s = t
            mask_entry_set.add((1, int(511 - t)))    # bwd step s = 511 - t
        per_core.append((tblp, idx_np, mask_c))

    mask_entries = tuple(sorted(mask_entry_set))
    nmask = max(1, len(mask_entries))

    in_maps = []
    for c in range(NCORES):
        tblp, idx_np, mask_c = per_core[c]
        msk = np.ones((128, 4 * nmask), np.float32)
        for mi, (d, s) in enumerate(mask_entries):
            t = s if d == 0 else 511 - s
            msk[:, 4 * mi:4 * mi + 4] = mask_c[:, t].astype(np.float32)[None, :]
        m = dict(tbl=tblp, idx=idx_np, ident=ident, w0=w0, r0=r0, w1=w1, r1=r1,
                 b0=b0, b1=b1, clsw=clsw, msk=msk)
        if has_clsb:
            m["clsb"] = np.broadcast_to(
                clsb_np.astype(np.float32), (128, NCLS)).copy()
        in_maps.append(m)
    return in_maps, mask_entries, has_clsb


# ---------------------------------------------------------------------------
def kernel(**inputs):
    from concourse.bass_utils import run_bass_kernel_spmd

    in_maps, mask_entries, has_clsb = _prep_host(inputs)

    key = (mask_entries, has_clsb, VARIANT)
    if key not in _prog_cache:
        _prog_cache[key] = _build_program(mask_entries, has_clsb,
                                          variant=VARIANT)
    nc = _prog_cache[key]

    res = run_bass_kernel_spmd(nc, in_maps, core_ids=list(range(NCORES)))

    out = np.empty((B, T, NCLS), np.float32)
    for c in range(NCORES):
        oc = res.results[c]["out"].reshape(T, BL, NCLS)
        out[BL * c:BL * (c + 1)] = oc.transpose(1, 0, 2)
    return out

